# revision 21
# baseline (speedup 1.0000x reference)
"""Causal multi-head self-attention (RoPE) on 8 TRN2 NeuronCores.

Sharding: core c = (batch b = c//2, head-group g = c%2). Each core computes
QKV projections for its 8 heads on its batch, RoPE, causal attention in
transposed-score space (scores^T = [k_part, q_free]; softmax sums via a
ones-column appended to V), a partial out-projection over its 512 head dims,
then a pairwise AllReduce [[0,1],[2,3],[4,5],[6,7]] sums the two head-group
partials into the full output.

Shapes (hardcoded): x [4, 2048, 1024], Wq/Wk/Wv/Wo [1024, 1024],
token_positions [2048]. D_K=64, N_HEADS=16, THETA=10000.

All matmuls run as float32r (full-rate fp32 mode on the PE). Host work is
layout-only: slicing, transposes, RoPE cos/sin tables, masks.
"""
import copy
import sys

sys.path.insert(0, "/opt/trn_rl_repo")

import numpy as np

import bass_rust
import concourse.bass as bass
import concourse.mybir as mybir
import concourse.tile as tile
from concourse.bass_utils import run_bass_kernel_spmd

P = 128
S = 2048
D = 1024
OG = 512          # head dims per group (8 heads x 64)
DK = 64
THETA = 10000.0
F32 = mybir.dt.float32
F32R = mybir.dt.float32r
NEG = -1.0e30

_cache = {}


def _split_multi_waits(nc, max_waits=1):
    """The staged walrus build rejects instructions carrying more than one
    attached sem-wait ("Too many sync wait commands"). Hoist excess waits
    into standalone single-wait EventSemaphore instructions just before the
    offending instruction (same engine, so semantics are identical)."""
    n_split = 0
    new_module = copy.replace(nc.m, functions=[])
    for function in nc.m.functions:
        new_function = copy.replace(function, blocks=[])
        new_function.set_allocations_from_list(function.allocations)
        for block in function.blocks:
            new_insts = []
            for inst in block.instructions:
                si = inst.sync_info
                if si is not None and len(si.on_wait) > max_waits:
                    waits = list(si.on_wait)
                    for j, w in enumerate(waits[:-max_waits]):
                        ev = bass_rust.InstEventSemaphore(
                            name=f"{inst.name}-wsplit{j}", ins=[], outs=[]
                        )
                        ev.engine = inst.engine
                        ev.sync_info = bass_rust.SyncInfo(on_wait=[w], on_update=[])
                        new_insts.append(ev)
                        n_split += 1
                    si.on_wait = waits[-max_waits:]
                new_insts.append(inst)
            new_block = copy.replace(block, instructions=new_insts)
            new_function.blocks.append(new_block)
        new_module.functions.append(new_function)
    nc.m = new_module
    return n_split


def _build_nc(split_waits=True):
    nc = bass.Bass(num_devices=8)

    xt_e = nc.declare_dram_parameter("xt", [D, S], F32R, isOutput=False)
    wq_e = nc.declare_dram_parameter("wqt", [D, OG], F32R, isOutput=False)
    wk_e = nc.declare_dram_parameter("wkt", [D, OG], F32R, isOutput=False)
    wv_e = nc.declare_dram_parameter("wvt", [D, OG], F32R, isOutput=False)
    wo_e = nc.declare_dram_parameter("wot", [OG, D], F32R, isOutput=False)
    cos_e = nc.declare_dram_parameter("cosT", [P, S], F32, isOutput=False)
    sin_e = nc.declare_dram_parameter("sinT", [P, S], F32, isOutput=False)
    mb_e = nc.declare_dram_parameter("mb", [P, 4, 512], F32R, isOutput=False)
    id_e = nc.declare_dram_parameter("ident", [P, P], F32R, isOutput=False)
    y_ext = nc.declare_dram_parameter("y", [S, D], F32, isOutput=True)

    y_int = nc.dram_tensor("y_int", [S, D], F32)
    y_ar = nc.dram_tensor("y_ar", [S, D], F32)

    ctx = tile.TileContext(nc)
    with ctx as tc, tc.tile_pool(name="persist", bufs=1) as persist:
        qkraw = persist.tile([P, 8, S], F32, tag="qkraw")  # q blocks 0-3, k blocks 4-7
        v_ext = persist.tile([P, 16, 8, 66], F32R)
        # col 64 of every (sb, h) slot must be 1.0 (softmax-sum ones column);
        # cols 0-63 are overwritten by the V-projection copies, col 65 unused.
        vcol = persist.tile([P, 1], F32)
        nc.vector.memset(vcol, 1.0)
        nc.vector.tensor_copy(
            v_ext[:, :, :, 64:65],
            vcol[:, None, None, :].to_broadcast((P, 16, 8, 1)),
        )

        # ---- projections: Q, K (transposed out: [dims, S]), then V ----
        with tc.tile_pool(name="xtp", bufs=1) as xtp, \
             tc.tile_pool(name="wpool", bufs=1) as wpool, \
             tc.tile_pool(name="prps", bufs=4, space="PSUM") as prps:
            xt = xtp.tile([P, 8, S], F32R)
            nc.sync.dma_start(
                out=xt, in_=xt_e.ap().rearrange("(dc p) s -> p dc s", p=P)
            )
            for w_ext, dst_base in [(wq_e, 0), (wk_e, 4)]:
                w_sb = wpool.tile([P, 8, OG], F32R, tag="w")
                nc.sync.dma_start(
                    out=w_sb, in_=w_ext.ap().rearrange("(dc p) o -> p dc o", p=P)
                )
                for ob in range(4):
                    for scn in range(4):
                        ps = prps.tile([P, 512], F32, tag="projps")
                        for dc in range(8):
                            nc.tensor.matmul(
                                ps,
                                w_sb[:, dc, ob * P:(ob + 1) * P],
                                xt[:, dc, scn * 512:(scn + 1) * 512],
                                start=(dc == 0),
                                stop=(dc == 7),
                            )
                        nc.vector.tensor_copy(
                            qkraw[:, dst_base + ob, scn * 512:(scn + 1) * 512]
                            .bitcast(F32R),
                            ps,
                        )
            # V projection: natural layout [s, dims] -> v_ext[:, sb, h, 0:64]
            w_sb = wpool.tile([P, 8, OG], F32R, tag="w")
            nc.sync.dma_start(
                out=w_sb, in_=wv_e.ap().rearrange("(dc p) o -> p dc o", p=P)
            )
            for sb in range(16):
                ps = prps.tile([P, 512], F32, tag="projps")
                for dc in range(8):
                    nc.tensor.matmul(
                        ps,
                        xt[:, dc, sb * P:(sb + 1) * P],
                        w_sb[:, dc, :],
                        start=(dc == 0),
                        stop=(dc == 7),
                    )
                nc.vector.tensor_copy(
                    v_ext[:, sb, :, 0:64],
                    ps.rearrange("p (h d) -> p h d", h=8),
                )

        # ---- RoPE on Q and K (in place; rotated output viewed as f32r) ----
        with tc.tile_pool(name="trig", bufs=1) as trig, \
             tc.tile_pool(name="rtmp", bufs=1) as rtmp:
            cos_s = trig.tile([P, S], F32)
            sin_s = trig.tile([P, S], F32)
            nc.sync.dma_start(out=cos_s, in_=cos_e.ap())
            nc.sync.dma_start(out=sin_s, in_=sin_e.ap())
            for pb in (0, 2, 4, 6):
                e_blk = qkraw[:, pb, :]
                o_blk = qkraw[:, pb + 1, :]
                t0 = rtmp.tile([P, S], F32, tag="t0")
                t1 = rtmp.tile([P, S], F32, tag="t1")
                t2 = rtmp.tile([P, S], F32, tag="t2")
                nc.vector.tensor_mul(t0, e_blk, cos_s)
                nc.vector.tensor_mul(t1, o_blk, sin_s)
                nc.vector.tensor_mul(t2, e_blk, sin_s)
                # e' = e*cos - o*sin ; o' = e*sin + o*cos  (in place)
                nc.vector.tensor_mul(o_blk.bitcast(F32R), o_blk, cos_s)  # := o*cos
                nc.vector.tensor_sub(e_blk.bitcast(F32R), t0, t1)
                nc.vector.tensor_add(o_blk.bitcast(F32R), t2, o_blk)
        rq = qkraw.bitcast(F32R)  # [:, 0:4] = rotated Q^T, [:, 4:8] = rotated K^T

        # ---- attention + incremental out-projection ----
        with tc.tile_pool(name="small", bufs=1) as small, \
             tc.tile_pool(name="wo_pool", bufs=1) as wo_pool, \
             tc.tile_pool(name="ppool", bufs=3) as ppool, \
             tc.tile_pool(name="opool", bufs=2) as opool, \
             tc.tile_pool(name="stg", bufs=4) as stgp, \
             tc.tile_pool(name="rbcp", bufs=2) as rbcp, \
             tc.tile_pool(name="yout", bufs=3) as yout, \
             tc.tile_pool(name="sps", bufs=2, space="PSUM") as spsp, \
             tc.tile_pool(name="avps", bufs=2, space="PSUM") as avpsp, \
             tc.tile_pool(name="yps", bufs=2, space="PSUM") as ypsp, \
             tc.tile_pool(name="rbdram", bufs=2, space="DRAM") as rbdram:
            mb_s = small.tile([P, 4, 512], F32R)
            nc.sync.dma_start(out=mb_s, in_=mb_e.ap())
            id_s = small.tile([P, P], F32R)
            nc.sync.dma_start(out=id_s, in_=id_e.ap())
            wo_s = wo_pool.tile([P, 4, D], F32R)
            nc.sync.dma_start(
                out=wo_s, in_=wo_e.ap().rearrange("(dc p) o -> p dc o", p=P)
            )

            for scn in range(4):
                o_chunk = opool.tile([P, 4, 512], F32R, tag="ochunk")
                nk = 4 * (scn + 1)
                for pr in range(4):
                    h0 = 2 * pr
                    h1 = 2 * pr + 1
                    av0 = avpsp.tile([65, 512], F32, tag="av", name="av0")
                    av1 = avpsp.tile([65, 512], F32, tag="av", name="av1")
                    av = [av0, av1]
                    for kb in range(nk):
                        sps = spsp.tile([P, 2, 512], F32, tag="sps")
                        diag = kb >= 4 * scn
                        for hh, h in enumerate((h0, h1)):
                            be = 2 * (h // 4)
                            row = 32 * (h % 4)
                            for j, bb in enumerate((be, be + 1)):
                                nc.tensor.matmul(
                                    sps[:, hh, :],
                                    rq[row:row + 32, 4 + bb, kb * P:(kb + 1) * P],
                                    rq[row:row + 32, bb, scn * 512:(scn + 1) * 512],
                                    start=(j == 0),
                                    stop=(j == 1 and not diag),
                                    tile_position=(row, 0),
                                )
                            if diag:
                                nc.tensor.matmul(
                                    sps[:, hh, :],
                                    id_s,
                                    mb_s[:, kb - 4 * scn, :],
                                    start=False,
                                    stop=True,
                                )
                        pt = ppool.tile([P, 2, 512], F32R, tag="pt")
                        nc.scalar.activation(
                            pt, sps, mybir.ActivationFunctionType.Exp, scale=0.125
                        )
                        for hh, h in enumerate((h0, h1)):
                            nc.tensor.matmul(
                                av[hh],
                                v_ext[:, kb, h, 0:65],
                                pt[:, hh, :],
                                start=(kb == 0),
                                stop=(kb == nk - 1),
                            )
                    # normalize: o_chunk[:, pr] rows 0-63 = head h0, 64-127 = h1
                    stg = stgp.tile([1, 1024], F32, tag="stg")
                    nc.vector.reciprocal(stg[0:1, 0:512], av[0][64:65, :])
                    nc.vector.reciprocal(stg[0:1, 512:1024], av[1][64:65, :])
                    # broadcast the two recip rows across 64 partitions each:
                    # bounce through DRAM, read back with a stride-0 leading
                    # dim (partition-broadcast DMA)
                    rb_d = rbdram.tile([1, 1024], F32, tag="rbd")
                    nc.sync.dma_start(out=rb_d, in_=stg)
                    rbc_sb = rbcp.tile([P, 512], F32, tag="rbcsb")
                    nc.sync.dma_start(
                        out=rbc_sb[0:64, :],
                        in_=rb_d[0:1, 0:512].to_broadcast((64, 512)),
                    )
                    nc.sync.dma_start(
                        out=rbc_sb[64:128, :],
                        in_=rb_d[0:1, 512:1024].to_broadcast((64, 512)),
                    )
                    nc.vector.tensor_mul(
                        o_chunk[0:64, pr, :], av[0][0:64, :], rbc_sb[0:64, :]
                    )
                    nc.vector.tensor_mul(
                        o_chunk[64:128, pr, :], av[1][0:64, :], rbc_sb[64:128, :]
                    )
                # out-projection for this q-chunk
                for qb in range(4):
                    for oc in range(2):
                        yps = ypsp.tile([P, 512], F32, tag="yps")
                        for db in range(4):
                            nc.tensor.matmul(
                                yps,
                                o_chunk[:, db, qb * P:(qb + 1) * P],
                                wo_s[:, db, oc * 512:(oc + 1) * 512],
                                start=(db == 0),
                                stop=(db == 3),
                            )
                        yt = yout.tile([P, 512], F32, tag="yt")
                        nc.vector.tensor_copy(yt, yps)
                        nc.sync.dma_start(
                            out=y_int[
                                scn * 512 + qb * P:scn * 512 + (qb + 1) * P,
                                oc * 512:(oc + 1) * 512,
                            ],
                            in_=yt,
                        )

    # ---- pairwise AllReduce of the partial outputs, then write y ----
    with (
        nc.Block() as block,
        nc.semaphore("cc_sem") as cc_sem,
        nc.semaphore("dma_sem") as dma_sem,
    ):
        @block.gpsimd
        def _(g):
            g.collective_compute(
                "AllReduce",
                mybir.AluOpType.add,
                replica_groups=[[0, 1], [2, 3], [4, 5], [6, 7]],
                ins=[y_int.ap().opt()],
                outs=[y_ar.ap().opt()],
            ).then_inc(cc_sem, 1)
            g.wait_ge(cc_sem, 1)
            g.dma_start(out=y_ext.ap(), in_=y_ar.ap()).then_inc(dma_sem, 16)
            g.wait_ge(dma_sem, 16)

    if split_waits:
        _split_multi_waits(nc)
    return nc


def _host_inputs(x, Wq, Wk, Wv, Wo, token_positions):
    """Per-core input dicts. Host work is layout-only (transpose/slice/tables)."""
    B = x.shape[0]
    half = DK // 2
    k = np.arange(1, half + 1, dtype=np.float64)
    inv_freq = THETA ** (-(2.0 * k - 2.0) / DK)  # [32]
    pos = np.asarray(token_positions).astype(np.float64)  # [S]
    ang = pos[None, :] * inv_freq[:, None]  # [32, S]
    cos32 = np.cos(ang).astype(np.float32)
    sin32 = np.sin(ang).astype(np.float32)
    cosT = np.ascontiguousarray(np.tile(cos32, (4, 1)))  # [128, S]
    sinT = np.ascontiguousarray(np.tile(sin32, (4, 1)))

    # permuted Q/K dim order within a group: blk(4) x h4(4) x k(32)
    perm = np.empty(OG, dtype=np.int64)
    i = 0
    for blk in range(4):
        for h4 in range(4):
            for kk in range(32):
                perm[i] = 64 * (4 * (blk // 2) + h4) + 2 * kk + (blk % 2)
                i += 1

    pp = np.arange(P)[:, None]
    ff = np.arange(512)[None, :]
    mb = np.empty((P, 4, 512), dtype=np.float32)
    for r in range(4):
        mb[:, r, :] = np.where(ff >= 128 * r + pp, 0.0, NEG)
    ident = np.eye(P, dtype=np.float32)

    in_maps = []
    for c in range(8):
        b = c // 2
        g = c % 2
        gd = slice(g * OG, (g + 1) * OG)  # group's head dims among 1024
        gdim = np.arange(g * OG, (g + 1) * OG)
        xt = np.ascontiguousarray(x[b].T.astype(np.float32))  # [D, S]
        wqt = np.ascontiguousarray(Wq[gdim[perm], :].T.astype(np.float32))
        wkt = np.ascontiguousarray(Wk[gdim[perm], :].T.astype(np.float32))
        wvt = np.ascontiguousarray(Wv[gd, :].T.astype(np.float32))
        wot = np.ascontiguousarray(Wo[:, gd].T.astype(np.float32))
        in_maps.append({
            "xt": xt, "wqt": wqt, "wkt": wkt, "wvt": wvt, "wot": wot,
            "cosT": cosT, "sinT": sinT, "mb": mb, "ident": ident,
        })
    return in_maps


def kernel(x, Wq, Wk, Wv, Wo, token_positions, _trace=False):
    if "nc" not in _cache:
        _cache["nc"] = _build_nc()
    nc = _cache["nc"]
    in_maps = _host_inputs(x, Wq, Wk, Wv, Wo, token_positions)
    res = run_bass_kernel_spmd(
        nc, in_maps, core_ids=list(range(8)), trace=_trace
    )
    _cache["last_result"] = res
    out = np.empty((x.shape[0], S, D), dtype=np.float32)
    for b in range(x.shape[0]):
        out[b] = res.results[2 * b]["y"]
    return out


# revision 23
# speedup vs baseline: 1.2920x; 1.2920x over previous
"""Causal multi-head self-attention (RoPE) on 8 TRN2 NeuronCores.

Sharding: core c = (batch b = c//2, head-group g = c%2). Each core computes
QKV projections for its 8 heads on its batch, RoPE, causal attention in
transposed-score space (scores^T = [k_part, q_free]; softmax sums via a
ones-column appended to V), a partial out-projection over its 512 head dims,
then a pairwise AllReduce [[0,1],[2,3],[4,5],[6,7]] sums the two head-group
partials into the full output.

Shapes (hardcoded): x [4, 2048, 1024], Wq/Wk/Wv/Wo [1024, 1024],
token_positions [2048]. D_K=64, N_HEADS=16, THETA=10000.

Matmul operands are bf16 (PSUM accumulation in fp32); softmax statistics and
the final output stay fp32. Host work is layout-only: slicing, transposes,
RoPE cos/sin tables, masks.
"""
import copy
import sys

sys.path.insert(0, "/opt/trn_rl_repo")

import ml_dtypes
import numpy as np

import bass_rust
import concourse.bass as bass
import concourse.mybir as mybir
import concourse.tile as tile
from concourse.bass_utils import run_bass_kernel_spmd

P = 128
S = 2048
D = 1024
OG = 512          # head dims per group (8 heads x 64)
DK = 64
THETA = 10000.0
F32 = mybir.dt.float32
BF16 = mybir.dt.bfloat16
NEG = -1.0e30
BF = ml_dtypes.bfloat16

_cache = {}


def _split_multi_waits(nc, max_waits=1):
    """The staged walrus build rejects instructions carrying more than one
    attached sem-wait ("Too many sync wait commands"). Hoist excess waits
    into standalone single-wait EventSemaphore instructions just before the
    offending instruction (same engine, so semantics are identical)."""
    n_split = 0
    new_module = copy.replace(nc.m, functions=[])
    for function in nc.m.functions:
        new_function = copy.replace(function, blocks=[])
        new_function.set_allocations_from_list(function.allocations)
        for block in function.blocks:
            new_insts = []
            for inst in block.instructions:
                si = inst.sync_info
                if si is not None and len(si.on_wait) > max_waits:
                    waits = list(si.on_wait)
                    for j, w in enumerate(waits[:-max_waits]):
                        ev = bass_rust.InstEventSemaphore(
                            name=f"{inst.name}-wsplit{j}", ins=[], outs=[]
                        )
                        ev.engine = inst.engine
                        ev.sync_info = bass_rust.SyncInfo(on_wait=[w], on_update=[])
                        new_insts.append(ev)
                        n_split += 1
                    si.on_wait = waits[-max_waits:]
                new_insts.append(inst)
            new_block = copy.replace(block, instructions=new_insts)
            new_function.blocks.append(new_block)
        new_module.functions.append(new_function)
    nc.m = new_module
    return n_split


def _build_nc(split_waits=True):
    nc = bass.Bass(num_devices=8)

    xt_e = nc.declare_dram_parameter("xt", [D, S], BF16, isOutput=False)
    wq_e = nc.declare_dram_parameter("wqt", [D, OG], BF16, isOutput=False)
    wk_e = nc.declare_dram_parameter("wkt", [D, OG], BF16, isOutput=False)
    wv_e = nc.declare_dram_parameter("wvt", [D, OG], BF16, isOutput=False)
    wo_e = nc.declare_dram_parameter("wot", [OG, D], BF16, isOutput=False)
    cos_e = nc.declare_dram_parameter("cosT", [P, S], BF16, isOutput=False)
    sin_e = nc.declare_dram_parameter("sinT", [P, S], BF16, isOutput=False)
    mb_e = nc.declare_dram_parameter("mb", [P, 4, 512], BF16, isOutput=False)
    id_e = nc.declare_dram_parameter("ident", [P, P], BF16, isOutput=False)
    y_ext = nc.declare_dram_parameter("y", [S, D], F32, isOutput=True)

    y_int = nc.dram_tensor("y_int", [S, D], F32)
    y_ar = nc.dram_tensor("y_ar", [S, D], F32)

    ctx = tile.TileContext(nc)
    with ctx as tc, tc.tile_pool(name="persist", bufs=1) as persist:
        qkraw = persist.tile([P, 8, S], BF16, tag="qkraw")  # Q blocks 0-3, K 4-7
        v_ext = persist.tile([P, 16, 8, 66], BF16)
        # col 64 of every (sb, h) slot must be 1.0 (softmax-sum ones column);
        # cols 0-63 are overwritten by the V-projection copies, col 65 unused.
        vcol = persist.tile([P, 1], BF16)
        nc.vector.memset(vcol, 1.0)
        nc.vector.tensor_copy(
            v_ext[:, :, :, 64:65],
            vcol[:, None, None, :].to_broadcast((P, 16, 8, 1)),
        )

        # ---- projections: Q, K (transposed out: [dims, S]), then V ----
        with tc.tile_pool(name="xtp", bufs=1) as xtp, \
             tc.tile_pool(name="wpool", bufs=2) as wpool, \
             tc.tile_pool(name="prps", bufs=4, space="PSUM") as prps:
            xt = xtp.tile([P, 8, S], BF16)
            nc.sync.dma_start(
                out=xt, in_=xt_e.ap().rearrange("(dc p) s -> p dc s", p=P)
            )
            for w_ext, dst_base in [(wq_e, 0), (wk_e, 4)]:
                w_sb = wpool.tile([P, 8, OG], BF16, tag="w")
                nc.sync.dma_start(
                    out=w_sb, in_=w_ext.ap().rearrange("(dc p) o -> p dc o", p=P)
                )
                for ob in range(4):
                    for scn in range(4):
                        ps = prps.tile([P, 512], F32, tag="projps")
                        for dc in range(8):
                            nc.tensor.matmul(
                                ps,
                                w_sb[:, dc, ob * P:(ob + 1) * P],
                                xt[:, dc, scn * 512:(scn + 1) * 512],
                                start=(dc == 0),
                                stop=(dc == 7),
                            )
                        nc.vector.tensor_copy(
                            qkraw[:, dst_base + ob, scn * 512:(scn + 1) * 512],
                            ps,
                        )
            # V projection: natural layout [s, dims] -> v_ext[:, sb, h, 0:64]
            w_sb = wpool.tile([P, 8, OG], BF16, tag="w")
            nc.sync.dma_start(
                out=w_sb, in_=wv_e.ap().rearrange("(dc p) o -> p dc o", p=P)
            )
            for sb in range(16):
                ps = prps.tile([P, 512], F32, tag="projps")
                for dc in range(8):
                    nc.tensor.matmul(
                        ps,
                        xt[:, dc, sb * P:(sb + 1) * P],
                        w_sb[:, dc, :],
                        start=(dc == 0),
                        stop=(dc == 7),
                    )
                nc.vector.tensor_copy(
                    v_ext[:, sb, :, 0:64],
                    ps.rearrange("p (h d) -> p h d", h=8),
                )

        # ---- RoPE on Q and K (in place) ----
        with tc.tile_pool(name="trig", bufs=1) as trig, \
             tc.tile_pool(name="rtmp", bufs=1) as rtmp:
            cos_s = trig.tile([P, S], BF16)
            sin_s = trig.tile([P, S], BF16)
            nc.sync.dma_start(out=cos_s, in_=cos_e.ap())
            nc.sync.dma_start(out=sin_s, in_=sin_e.ap())
            for pb in (0, 2, 4, 6):
                e_blk = qkraw[:, pb, :]
                o_blk = qkraw[:, pb + 1, :]
                t0 = rtmp.tile([P, S], BF16, tag="t0")
                t1 = rtmp.tile([P, S], BF16, tag="t1")
                t2 = rtmp.tile([P, S], BF16, tag="t2")
                nc.vector.tensor_mul(t0, e_blk, cos_s)
                nc.vector.tensor_mul(t1, o_blk, sin_s)
                nc.vector.tensor_mul(t2, e_blk, sin_s)
                # e' = e*cos - o*sin ; o' = e*sin + o*cos  (in place)
                nc.vector.tensor_mul(o_blk, o_blk, cos_s)  # o_blk := o*cos
                nc.vector.tensor_sub(e_blk, t0, t1)
                nc.vector.tensor_add(o_blk, t2, o_blk)
        rq = qkraw  # [:, 0:4] = rotated Q^T, [:, 4:8] = rotated K^T

        # ---- attention + incremental out-projection ----
        with tc.tile_pool(name="small", bufs=1) as small, \
             tc.tile_pool(name="wo_pool", bufs=1) as wo_pool, \
             tc.tile_pool(name="ppool", bufs=4) as ppool, \
             tc.tile_pool(name="opool", bufs=2) as opool, \
             tc.tile_pool(name="stg", bufs=4) as stgp, \
             tc.tile_pool(name="rbcp", bufs=2) as rbcp, \
             tc.tile_pool(name="yout", bufs=3) as yout, \
             tc.tile_pool(name="sps", bufs=2, space="PSUM") as spsp, \
             tc.tile_pool(name="avps", bufs=2, space="PSUM") as avpsp, \
             tc.tile_pool(name="yps", bufs=2, space="PSUM") as ypsp, \
             tc.tile_pool(name="rbdram", bufs=2, space="DRAM") as rbdram:
            mb_s = small.tile([P, 4, 512], BF16)
            nc.sync.dma_start(out=mb_s, in_=mb_e.ap())
            id_s = small.tile([P, P], BF16)
            nc.sync.dma_start(out=id_s, in_=id_e.ap())
            wo_s = wo_pool.tile([P, 4, D], BF16)
            nc.sync.dma_start(
                out=wo_s, in_=wo_e.ap().rearrange("(dc p) o -> p dc o", p=P)
            )

            for scn in range(4):
                o_chunk = opool.tile([P, 4, 512], BF16, tag="ochunk")
                nk = 4 * (scn + 1)
                for pr in range(4):
                    h0 = 2 * pr
                    h1 = 2 * pr + 1
                    av0 = avpsp.tile([65, 512], F32, tag="av", name="av0")
                    av1 = avpsp.tile([65, 512], F32, tag="av", name="av1")
                    av = [av0, av1]
                    for kb in range(nk):
                        sps = spsp.tile([P, 2, 512], F32, tag="sps")
                        diag = kb >= 4 * scn
                        for hh, h in enumerate((h0, h1)):
                            be = 2 * (h // 4)
                            row = 32 * (h % 4)
                            for j, bb in enumerate((be, be + 1)):
                                nc.tensor.matmul(
                                    sps[:, hh, :],
                                    rq[row:row + 32, 4 + bb, kb * P:(kb + 1) * P],
                                    rq[row:row + 32, bb, scn * 512:(scn + 1) * 512],
                                    start=(j == 0),
                                    stop=(j == 1 and not diag),
                                    tile_position=(row, 0),
                                )
                            if diag:
                                nc.tensor.matmul(
                                    sps[:, hh, :],
                                    id_s,
                                    mb_s[:, kb - 4 * scn, :],
                                    start=False,
                                    stop=True,
                                )
                        pt = ppool.tile([P, 2, 512], BF16, tag="pt")
                        nc.scalar.activation(
                            pt, sps, mybir.ActivationFunctionType.Exp, scale=0.125
                        )
                        for hh, h in enumerate((h0, h1)):
                            nc.tensor.matmul(
                                av[hh],
                                v_ext[:, kb, h, 0:65],
                                pt[:, hh, :],
                                start=(kb == 0),
                                stop=(kb == nk - 1),
                            )
                    # normalize: o_chunk[:, pr] rows 0-63 = head h0, 64-127 = h1
                    # 1/s as exp(-ln(s)) on the scalar engine: DVE reciprocal
                    # is an 8x-iterative op and costs 4.3us per 512-elem row
                    stg = stgp.tile([1, 1024], F32, tag="stg")
                    lns = stgp.tile([1, 1024], F32, tag="lns")
                    nc.scalar.activation(
                        lns[0:1, 0:512], av[0][64:65, :],
                        mybir.ActivationFunctionType.Ln,
                    )
                    nc.scalar.activation(
                        lns[0:1, 512:1024], av[1][64:65, :],
                        mybir.ActivationFunctionType.Ln,
                    )
                    nc.scalar.activation(
                        stg, lns, mybir.ActivationFunctionType.Exp, scale=-1.0
                    )
                    # broadcast the two recip rows across 64 partitions each:
                    # bounce through DRAM, read back with a stride-0 leading
                    # dim (partition-broadcast DMA)
                    rb_d = rbdram.tile([1, 1024], F32, tag="rbd")
                    nc.sync.dma_start(out=rb_d, in_=stg)
                    rbc_sb = rbcp.tile([P, 512], F32, tag="rbcsb")
                    nc.sync.dma_start(
                        out=rbc_sb[0:64, :],
                        in_=rb_d[0:1, 0:512].to_broadcast((64, 512)),
                    )
                    nc.sync.dma_start(
                        out=rbc_sb[64:128, :],
                        in_=rb_d[0:1, 512:1024].to_broadcast((64, 512)),
                    )
                    nc.vector.tensor_mul(
                        o_chunk[0:64, pr, :], av[0][0:64, :], rbc_sb[0:64, :]
                    )
                    nc.vector.tensor_mul(
                        o_chunk[64:128, pr, :], av[1][0:64, :], rbc_sb[64:128, :]
                    )
                # out-projection for this q-chunk
                for qb in range(4):
                    for oc in range(2):
                        yps = ypsp.tile([P, 512], F32, tag="yps")
                        for db in range(4):
                            nc.tensor.matmul(
                                yps,
                                o_chunk[:, db, qb * P:(qb + 1) * P],
                                wo_s[:, db, oc * 512:(oc + 1) * 512],
                                start=(db == 0),
                                stop=(db == 3),
                            )
                        yt = yout.tile([P, 512], F32, tag="yt")
                        nc.vector.tensor_copy(yt, yps)
                        nc.sync.dma_start(
                            out=y_int[
                                scn * 512 + qb * P:scn * 512 + (qb + 1) * P,
                                oc * 512:(oc + 1) * 512,
                            ],
                            in_=yt,
                        )

    # ---- pairwise AllReduce of the partial outputs, then write y ----
    with (
        nc.Block() as block,
        nc.semaphore("cc_sem") as cc_sem,
        nc.semaphore("dma_sem") as dma_sem,
    ):
        @block.gpsimd
        def _(g):
            g.collective_compute(
                "AllReduce",
                mybir.AluOpType.add,
                replica_groups=[[0, 1], [2, 3], [4, 5], [6, 7]],
                ins=[y_int.ap().opt()],
                outs=[y_ar.ap().opt()],
            ).then_inc(cc_sem, 1)
            g.wait_ge(cc_sem, 1)
            g.dma_start(out=y_ext.ap(), in_=y_ar.ap()).then_inc(dma_sem, 16)
            g.wait_ge(dma_sem, 16)

    if split_waits:
        _split_multi_waits(nc)
    return nc


def _host_inputs(x, Wq, Wk, Wv, Wo, token_positions):
    """Per-core input dicts. Host work is layout-only (transpose/slice/tables)."""
    half = DK // 2
    k = np.arange(1, half + 1, dtype=np.float64)
    inv_freq = THETA ** (-(2.0 * k - 2.0) / DK)  # [32]
    pos = np.asarray(token_positions).astype(np.float64)  # [S]
    ang = pos[None, :] * inv_freq[:, None]  # [32, S]
    cosT = np.ascontiguousarray(np.tile(np.cos(ang), (4, 1))).astype(BF)
    sinT = np.ascontiguousarray(np.tile(np.sin(ang), (4, 1))).astype(BF)

    # permuted Q/K dim order within a group: blk(4) x h4(4) x k(32);
    # blk 0/1 = heads 0-3 evens/odds, blk 2/3 = heads 4-7 evens/odds
    perm = np.empty(OG, dtype=np.int64)
    i = 0
    for blk in range(4):
        for h4 in range(4):
            for kk in range(32):
                perm[i] = 64 * (4 * (blk // 2) + h4) + 2 * kk + (blk % 2)
                i += 1

    pp = np.arange(P)[:, None]
    ff = np.arange(512)[None, :]
    mb = np.empty((P, 4, 512), dtype=np.float32)
    for r in range(4):
        mb[:, r, :] = np.where(ff >= 128 * r + pp, 0.0, NEG)
    mb = mb.astype(BF)
    ident = np.eye(P, dtype=np.float32).astype(BF)

    in_maps = []
    for c in range(8):
        b = c // 2
        g = c % 2
        gd = slice(g * OG, (g + 1) * OG)  # group's head dims among 1024
        gdim = np.arange(g * OG, (g + 1) * OG)
        xt = np.ascontiguousarray(x[b].T).astype(BF)  # [D, S]
        wqt = np.ascontiguousarray(Wq[gdim[perm], :].T).astype(BF)
        wkt = np.ascontiguousarray(Wk[gdim[perm], :].T).astype(BF)
        wvt = np.ascontiguousarray(Wv[gd, :].T).astype(BF)
        wot = np.ascontiguousarray(Wo[:, gd].T).astype(BF)
        in_maps.append({
            "xt": xt, "wqt": wqt, "wkt": wkt, "wvt": wvt, "wot": wot,
            "cosT": cosT, "sinT": sinT, "mb": mb, "ident": ident,
        })
    return in_maps


def kernel(x, Wq, Wk, Wv, Wo, token_positions, _trace=False):
    if "nc" not in _cache:
        _cache["nc"] = _build_nc()
    nc = _cache["nc"]
    in_maps = _host_inputs(x, Wq, Wk, Wv, Wo, token_positions)
    res = run_bass_kernel_spmd(
        nc, in_maps, core_ids=list(range(8)), trace=_trace
    )
    _cache["last_result"] = res
    out = np.empty((x.shape[0], S, D), dtype=np.float32)
    for b in range(x.shape[0]):
        out[b] = res.results[2 * b]["y"]
    return out


# revision 25
# speedup vs baseline: 1.5032x; 1.1634x over previous
"""Causal multi-head self-attention (RoPE) on 8 TRN2 NeuronCores.

Sharding: core c = (batch b = c//2, head-group g = c%2). Each core computes
QKV projections for its 8 heads on its batch, RoPE, causal attention in
transposed-score space (scores^T = [k_part, q_free]; softmax sums via a
ones-column appended to V), a partial out-projection over its 512 head dims,
then a pairwise AllReduce [[0,1],[2,3],[4,5],[6,7]] sums the two head-group
partials into the full output.

Shapes (hardcoded): x [4, 2048, 1024], Wq/Wk/Wv/Wo [1024, 1024],
token_positions [2048]. D_K=64, N_HEADS=16, THETA=10000.

Matmul operands are bf16 (PSUM accumulation in fp32); softmax statistics and
the final output stay fp32. Host work is layout-only: slicing, transposes,
RoPE cos/sin tables, masks.
"""
import copy
import sys

sys.path.insert(0, "/opt/trn_rl_repo")

import ml_dtypes
import numpy as np

import bass_rust
import concourse.bass as bass
import concourse.mybir as mybir
import concourse.tile as tile
from concourse.bass_utils import run_bass_kernel_spmd

P = 128
S = 2048
D = 1024
OG = 512          # head dims per group (8 heads x 64)
DK = 64
THETA = 10000.0
F32 = mybir.dt.float32
BF16 = mybir.dt.bfloat16
NEG = -1.0e30
BF = ml_dtypes.bfloat16

_cache = {}


def _split_multi_waits(nc, max_waits=1):
    """The staged walrus build rejects instructions carrying more than one
    attached sem-wait ("Too many sync wait commands"). Hoist excess waits
    into standalone single-wait EventSemaphore instructions just before the
    offending instruction (same engine, so semantics are identical)."""
    n_split = 0
    new_module = copy.replace(nc.m, functions=[])
    for function in nc.m.functions:
        new_function = copy.replace(function, blocks=[])
        new_function.set_allocations_from_list(function.allocations)
        for block in function.blocks:
            new_insts = []
            for inst in block.instructions:
                si = inst.sync_info
                if si is not None and len(si.on_wait) > max_waits:
                    waits = list(si.on_wait)
                    for j, w in enumerate(waits[:-max_waits]):
                        ev = bass_rust.InstEventSemaphore(
                            name=f"{inst.name}-wsplit{j}", ins=[], outs=[]
                        )
                        ev.engine = inst.engine
                        ev.sync_info = bass_rust.SyncInfo(on_wait=[w], on_update=[])
                        new_insts.append(ev)
                        n_split += 1
                    si.on_wait = waits[-max_waits:]
                new_insts.append(inst)
            new_block = copy.replace(block, instructions=new_insts)
            new_function.blocks.append(new_block)
        new_module.functions.append(new_function)
    nc.m = new_module
    return n_split


def _build_nc(split_waits=True):
    nc = bass.Bass(num_devices=8)

    xt_e = nc.declare_dram_parameter("xt", [D, S], BF16, isOutput=False)
    wq_e = nc.declare_dram_parameter("wqt", [D, OG], BF16, isOutput=False)
    wk_e = nc.declare_dram_parameter("wkt", [D, OG], BF16, isOutput=False)
    wv_e = nc.declare_dram_parameter("wvt", [D, OG], BF16, isOutput=False)
    wo_e = nc.declare_dram_parameter("wot", [OG, D], BF16, isOutput=False)
    cos_e = nc.declare_dram_parameter("cosT", [P, S], BF16, isOutput=False)
    sin_e = nc.declare_dram_parameter("sinT", [P, S], BF16, isOutput=False)
    mb_e = nc.declare_dram_parameter("mb", [P, 4, 512], BF16, isOutput=False)
    id_e = nc.declare_dram_parameter("ident", [P, P], BF16, isOutput=False)
    y_ext = nc.declare_dram_parameter("y", [S, D], F32, isOutput=True)

    ctx = tile.TileContext(nc)
    with ctx as tc, tc.tile_pool(name="persist", bufs=1) as persist:
        qkraw = persist.tile([P, 8, S], BF16, tag="qkraw")  # Q blocks 0-3, K 4-7
        v_ext = persist.tile([P, 16, 8, 66], BF16)
        # col 64 of every (sb, h) slot must be 1.0 (softmax-sum ones column);
        # cols 0-63 are overwritten by the V-projection copies, col 65 unused.
        vcol = persist.tile([P, 1], BF16)
        nc.vector.memset(vcol, 1.0)
        nc.vector.tensor_copy(
            v_ext[:, :, :, 64:65],
            vcol[:, None, None, :].to_broadcast((P, 16, 8, 1)),
        )

        # ---- projections: Q, K (transposed out: [dims, S]), then V ----
        with tc.tile_pool(name="xtp", bufs=1) as xtp, \
             tc.tile_pool(name="wpool", bufs=2) as wpool, \
             tc.tile_pool(name="prps", bufs=4, space="PSUM") as prps:
            xt = xtp.tile([P, 8, S], BF16)
            nc.sync.dma_start(
                out=xt, in_=xt_e.ap().rearrange("(dc p) s -> p dc s", p=P)
            )
            for w_ext, dst_base in [(wq_e, 0), (wk_e, 4)]:
                w_sb = wpool.tile([P, 8, OG], BF16, tag="w")
                nc.sync.dma_start(
                    out=w_sb, in_=w_ext.ap().rearrange("(dc p) o -> p dc o", p=P)
                )
                for ob in range(4):
                    for scn in range(4):
                        ps = prps.tile([P, 512], F32, tag="projps")
                        for dc in range(8):
                            nc.tensor.matmul(
                                ps,
                                w_sb[:, dc, ob * P:(ob + 1) * P],
                                xt[:, dc, scn * 512:(scn + 1) * 512],
                                start=(dc == 0),
                                stop=(dc == 7),
                            )
                        nc.vector.tensor_copy(
                            qkraw[:, dst_base + ob, scn * 512:(scn + 1) * 512],
                            ps,
                        )
            # V projection: natural layout [s, dims] -> v_ext[:, sb, h, 0:64]
            w_sb = wpool.tile([P, 8, OG], BF16, tag="w")
            nc.sync.dma_start(
                out=w_sb, in_=wv_e.ap().rearrange("(dc p) o -> p dc o", p=P)
            )
            for sb in range(16):
                ps = prps.tile([P, 512], F32, tag="projps")
                for dc in range(8):
                    nc.tensor.matmul(
                        ps,
                        xt[:, dc, sb * P:(sb + 1) * P],
                        w_sb[:, dc, :],
                        start=(dc == 0),
                        stop=(dc == 7),
                    )
                nc.vector.tensor_copy(
                    v_ext[:, sb, :, 0:64],
                    ps.rearrange("p (h d) -> p h d", h=8),
                )

        # ---- RoPE on Q and K (in place) ----
        with tc.tile_pool(name="trig", bufs=1) as trig, \
             tc.tile_pool(name="rtmp", bufs=1) as rtmp:
            cos_s = trig.tile([P, S], BF16)
            sin_s = trig.tile([P, S], BF16)
            nc.sync.dma_start(out=cos_s, in_=cos_e.ap())
            nc.sync.dma_start(out=sin_s, in_=sin_e.ap())
            for pb in (0, 2, 4, 6):
                e_blk = qkraw[:, pb, :]
                o_blk = qkraw[:, pb + 1, :]
                t0 = rtmp.tile([P, S], BF16, tag="t0")
                t1 = rtmp.tile([P, S], BF16, tag="t1")
                t2 = rtmp.tile([P, S], BF16, tag="t2")
                nc.vector.tensor_mul(t0, e_blk, cos_s)
                nc.vector.tensor_mul(t1, o_blk, sin_s)
                nc.vector.tensor_mul(t2, e_blk, sin_s)
                # e' = e*cos - o*sin ; o' = e*sin + o*cos  (in place)
                nc.vector.tensor_mul(o_blk, o_blk, cos_s)  # o_blk := o*cos
                nc.vector.tensor_sub(e_blk, t0, t1)
                nc.vector.tensor_add(o_blk, t2, o_blk)
        # ---- repack to head-contiguous layout: rqh blk t holds the head
        # pair (2t, 2t+1) as 64 contiguous rows each ([evens|odds] per head),
        # so score matmuls contract K=64 in one shot. 32 small SBUF->SBUF
        # DMAs, hidden under the projection/RoPE tail.
        rqh = persist.tile([P, 8, S], BF16, tag="rqh")
        for t in range(8):
            side = 4 * (t // 4)          # 0 = Q blocks, 4 = K blocks
            p_loc = t % 4                # head pair within side
            be = side + 2 * (p_loc // 2)
            bo = be + 1
            for hh in range(2):          # head within pair
                h = 2 * p_loc + hh
                row = 32 * (h % 4)
                nc.sync.dma_start(
                    out=rqh[64 * hh:64 * hh + 32, t, :],
                    in_=qkraw[row:row + 32, be, :],
                )
                nc.sync.dma_start(
                    out=rqh[64 * hh + 32:64 * hh + 64, t, :],
                    in_=qkraw[row:row + 32, bo, :],
                )

        # ---- attention + incremental out-projection ----
        with tc.tile_pool(name="small", bufs=1) as small, \
             tc.tile_pool(name="wo_pool", bufs=1) as wo_pool, \
             tc.tile_pool(name="ppool", bufs=4) as ppool, \
             tc.tile_pool(name="opool", bufs=2) as opool, \
             tc.tile_pool(name="stg", bufs=4) as stgp, \
             tc.tile_pool(name="rbcp", bufs=2) as rbcp, \
             tc.tile_pool(name="yout", bufs=3) as yout, \
             tc.tile_pool(name="sps", bufs=2, space="PSUM") as spsp, \
             tc.tile_pool(name="avps", bufs=2, space="PSUM") as avpsp, \
             tc.tile_pool(name="yps", bufs=2, space="PSUM") as ypsp, \
             tc.tile_pool(name="rbdram", bufs=2, space="DRAM") as rbdram, \
             tc.tile_pool(name="ardram", bufs=2, space="DRAM") as ardram:
            mb_s = small.tile([P, 4, 512], BF16)
            nc.sync.dma_start(out=mb_s, in_=mb_e.ap())
            id_s = small.tile([P, P], BF16)
            nc.sync.dma_start(out=id_s, in_=id_e.ap())
            wo_s = wo_pool.tile([P, 4, D], BF16)
            nc.sync.dma_start(
                out=wo_s, in_=wo_e.ap().rearrange("(dc p) o -> p dc o", p=P)
            )

            for scn in range(4):
                o_chunk = opool.tile([P, 4, 512], BF16, tag="ochunk")
                nk = 4 * (scn + 1)
                for pr in range(4):
                    h0 = 2 * pr
                    h1 = 2 * pr + 1
                    av0 = avpsp.tile([65, 512], F32, tag="av", name="av0")
                    av1 = avpsp.tile([65, 512], F32, tag="av", name="av1")
                    av = [av0, av1]
                    for kb in range(nk):
                        sps = spsp.tile([P, 2, 512], F32, tag="sps")
                        diag = kb >= 4 * scn
                        for hh in range(2):
                            rows = slice(64 * hh, 64 * hh + 64)
                            nc.tensor.matmul(
                                sps[:, hh, :],
                                rqh[rows, 4 + pr, kb * P:(kb + 1) * P],
                                rqh[rows, pr, scn * 512:(scn + 1) * 512],
                                start=True,
                                stop=not diag,
                                tile_position=(64 * hh, 0),
                            )
                            if diag:
                                nc.tensor.matmul(
                                    sps[:, hh, :],
                                    id_s,
                                    mb_s[:, kb - 4 * scn, :],
                                    start=False,
                                    stop=True,
                                )
                        pt = ppool.tile([P, 2, 512], BF16, tag="pt")
                        nc.scalar.activation(
                            pt, sps, mybir.ActivationFunctionType.Exp, scale=0.125
                        )
                        for hh, h in enumerate((h0, h1)):
                            nc.tensor.matmul(
                                av[hh],
                                v_ext[:, kb, h, 0:65],
                                pt[:, hh, :],
                                start=(kb == 0),
                                stop=(kb == nk - 1),
                            )
                    # normalize: o_chunk[:, pr] rows 0-63 = head h0, 64-127 = h1
                    # 1/s as exp(-ln(s)) on the scalar engine: DVE reciprocal
                    # is an 8x-iterative op and costs 4.3us per 512-elem row
                    stg = stgp.tile([1, 1024], F32, tag="stg")
                    lns = stgp.tile([1, 1024], F32, tag="lns")
                    nc.scalar.activation(
                        lns[0:1, 0:512], av[0][64:65, :],
                        mybir.ActivationFunctionType.Ln,
                    )
                    nc.scalar.activation(
                        lns[0:1, 512:1024], av[1][64:65, :],
                        mybir.ActivationFunctionType.Ln,
                    )
                    nc.scalar.activation(
                        stg, lns, mybir.ActivationFunctionType.Exp, scale=-1.0
                    )
                    # broadcast the two recip rows across 64 partitions each:
                    # bounce through DRAM, read back with a stride-0 leading
                    # dim (partition-broadcast DMA)
                    rb_d = rbdram.tile([1, 1024], F32, tag="rbd")
                    nc.sync.dma_start(out=rb_d, in_=stg)
                    rbc_sb = rbcp.tile([P, 512], F32, tag="rbcsb")
                    nc.sync.dma_start(
                        out=rbc_sb[0:64, :],
                        in_=rb_d[0:1, 0:512].to_broadcast((64, 512)),
                    )
                    nc.sync.dma_start(
                        out=rbc_sb[64:128, :],
                        in_=rb_d[0:1, 512:1024].to_broadcast((64, 512)),
                    )
                    nc.vector.tensor_mul(
                        o_chunk[0:64, pr, :], av[0][0:64, :], rbc_sb[0:64, :]
                    )
                    nc.vector.tensor_mul(
                        o_chunk[64:128, pr, :], av[1][0:64, :], rbc_sb[64:128, :]
                    )
                # out-projection for this q-chunk -> DRAM bounce, then a
                # pairwise AllReduce of just this chunk (overlaps the next
                # chunk's attention), then the final y write for these rows
                arin = ardram.tile([512, D], F32, tag="arin")
                arout = ardram.tile([512, D], F32, tag="arout")
                for qb in range(4):
                    for oc in range(2):
                        yps = ypsp.tile([P, 512], F32, tag="yps")
                        for db in range(4):
                            nc.tensor.matmul(
                                yps,
                                o_chunk[:, db, qb * P:(qb + 1) * P],
                                wo_s[:, db, oc * 512:(oc + 1) * 512],
                                start=(db == 0),
                                stop=(db == 3),
                            )
                        yt = yout.tile([P, 512], F32, tag="yt")
                        nc.vector.tensor_copy(yt, yps)
                        nc.sync.dma_start(
                            out=arin[qb * P:(qb + 1) * P, oc * 512:(oc + 1) * 512],
                            in_=yt,
                        )
                nc.gpsimd.collective_compute(
                    "AllReduce",
                    mybir.AluOpType.add,
                    replica_groups=[[0, 1], [2, 3], [4, 5], [6, 7]],
                    ins=[arin.opt()],
                    outs=[arout.opt()],
                )
                nc.sync.dma_start(
                    out=y_ext.ap()[scn * 512:(scn + 1) * 512, :], in_=arout
                )

    if split_waits:
        _split_multi_waits(nc)
    return nc


def _host_inputs(x, Wq, Wk, Wv, Wo, token_positions):
    """Per-core input dicts. Host work is layout-only (transpose/slice/tables)."""
    half = DK // 2
    k = np.arange(1, half + 1, dtype=np.float64)
    inv_freq = THETA ** (-(2.0 * k - 2.0) / DK)  # [32]
    pos = np.asarray(token_positions).astype(np.float64)  # [S]
    ang = pos[None, :] * inv_freq[:, None]  # [32, S]
    cosT = np.ascontiguousarray(np.tile(np.cos(ang), (4, 1))).astype(BF)
    sinT = np.ascontiguousarray(np.tile(np.sin(ang), (4, 1))).astype(BF)

    # permuted Q/K dim order within a group: blk(4) x h4(4) x k(32);
    # blk 0/1 = heads 0-3 evens/odds, blk 2/3 = heads 4-7 evens/odds
    perm = np.empty(OG, dtype=np.int64)
    i = 0
    for blk in range(4):
        for h4 in range(4):
            for kk in range(32):
                perm[i] = 64 * (4 * (blk // 2) + h4) + 2 * kk + (blk % 2)
                i += 1

    pp = np.arange(P)[:, None]
    ff = np.arange(512)[None, :]
    mb = np.empty((P, 4, 512), dtype=np.float32)
    for r in range(4):
        mb[:, r, :] = np.where(ff >= 128 * r + pp, 0.0, NEG)
    mb = mb.astype(BF)
    ident = np.eye(P, dtype=np.float32).astype(BF)

    in_maps = []
    for c in range(8):
        b = c // 2
        g = c % 2
        gd = slice(g * OG, (g + 1) * OG)  # group's head dims among 1024
        gdim = np.arange(g * OG, (g + 1) * OG)
        xt = np.ascontiguousarray(x[b].T).astype(BF)  # [D, S]
        wqt = np.ascontiguousarray(Wq[gdim[perm], :].T).astype(BF)
        wkt = np.ascontiguousarray(Wk[gdim[perm], :].T).astype(BF)
        wvt = np.ascontiguousarray(Wv[gd, :].T).astype(BF)
        wot = np.ascontiguousarray(Wo[:, gd].T).astype(BF)
        in_maps.append({
            "xt": xt, "wqt": wqt, "wkt": wkt, "wvt": wvt, "wot": wot,
            "cosT": cosT, "sinT": sinT, "mb": mb, "ident": ident,
        })
    return in_maps


def kernel(x, Wq, Wk, Wv, Wo, token_positions, _trace=False):
    if "nc" not in _cache:
        _cache["nc"] = _build_nc()
    nc = _cache["nc"]
    in_maps = _host_inputs(x, Wq, Wk, Wv, Wo, token_positions)
    res = run_bass_kernel_spmd(
        nc, in_maps, core_ids=list(range(8)), trace=_trace
    )
    _cache["last_result"] = res
    out = np.empty((x.shape[0], S, D), dtype=np.float32)
    for b in range(x.shape[0]):
        out[b] = res.results[2 * b]["y"]
    return out


# revision 26
# speedup vs baseline: 1.8279x; 1.2161x over previous
"""Causal multi-head self-attention (RoPE) on 8 TRN2 NeuronCores.

Sharding: core c = (batch b = c//2, head-group g = c%2). Each core computes
QKV projections for its 8 heads on its batch, RoPE, causal attention in
transposed-score space (scores^T = [k_part, q_free]; softmax sums via a
ones-column appended to V), a partial out-projection over its 512 head dims,
then a pairwise AllReduce [[0,1],[2,3],[4,5],[6,7]] sums the two head-group
partials into the full output.

Shapes (hardcoded): x [4, 2048, 1024], Wq/Wk/Wv/Wo [1024, 1024],
token_positions [2048]. D_K=64, N_HEADS=16, THETA=10000.

Matmul operands are bf16 (PSUM accumulation in fp32); softmax statistics and
the final output stay fp32. Host work is layout-only: slicing, transposes,
RoPE cos/sin tables, masks.
"""
import copy
import sys

sys.path.insert(0, "/opt/trn_rl_repo")

import ml_dtypes
import numpy as np

import bass_rust
import concourse.bass as bass
import concourse.mybir as mybir
import concourse.tile as tile
from concourse.bass_utils import run_bass_kernel_spmd

P = 128
S = 2048
D = 1024
OG = 512          # head dims per group (8 heads x 64)
DK = 64
THETA = 10000.0
F32 = mybir.dt.float32
BF16 = mybir.dt.bfloat16
NEG = -1.0e30
BF = ml_dtypes.bfloat16

_cache = {}


def _split_multi_waits(nc, max_waits=1):
    """The staged walrus build rejects instructions carrying more than one
    attached sem-wait ("Too many sync wait commands"). Hoist excess waits
    into standalone single-wait EventSemaphore instructions just before the
    offending instruction (same engine, so semantics are identical)."""
    n_split = 0
    new_module = copy.replace(nc.m, functions=[])
    for function in nc.m.functions:
        new_function = copy.replace(function, blocks=[])
        new_function.set_allocations_from_list(function.allocations)
        for block in function.blocks:
            new_insts = []
            for inst in block.instructions:
                si = inst.sync_info
                if si is not None and len(si.on_wait) > max_waits:
                    waits = list(si.on_wait)
                    for j, w in enumerate(waits[:-max_waits]):
                        ev = bass_rust.InstEventSemaphore(
                            name=f"{inst.name}-wsplit{j}", ins=[], outs=[]
                        )
                        ev.engine = inst.engine
                        ev.sync_info = bass_rust.SyncInfo(on_wait=[w], on_update=[])
                        new_insts.append(ev)
                        n_split += 1
                    si.on_wait = waits[-max_waits:]
                new_insts.append(inst)
            new_block = copy.replace(block, instructions=new_insts)
            new_function.blocks.append(new_block)
        new_module.functions.append(new_function)
    nc.m = new_module
    return n_split


def _build_nc(split_waits=True):
    nc = bass.Bass(num_devices=8)

    xt_e = nc.declare_dram_parameter("xt", [D, S], BF16, isOutput=False)
    wq_e = nc.declare_dram_parameter("wqt", [D, OG], BF16, isOutput=False)
    wk_e = nc.declare_dram_parameter("wkt", [D, OG], BF16, isOutput=False)
    wv_e = nc.declare_dram_parameter("wvt", [D, OG], BF16, isOutput=False)
    wo_e = nc.declare_dram_parameter("wot", [OG, D], BF16, isOutput=False)
    cos_e = nc.declare_dram_parameter("cosT", [P, S], BF16, isOutput=False)
    sin_e = nc.declare_dram_parameter("sinT", [P, S], BF16, isOutput=False)
    mb_e = nc.declare_dram_parameter("mb", [P, 4, 512], BF16, isOutput=False)
    id_e = nc.declare_dram_parameter("ident", [P, P], BF16, isOutput=False)
    y_ext = nc.declare_dram_parameter("y", [S, D], F32, isOutput=True)

    ctx = tile.TileContext(nc)
    with ctx as tc, tc.tile_pool(name="persist", bufs=1) as persist:
        qkraw = persist.tile([P, 8, S], BF16, tag="qkraw")  # Q blocks 0-3, K 4-7
        v_ext = persist.tile([P, 16, 8, 66], BF16)
        # col 64 of every (sb, h) slot must be 1.0 (softmax-sum ones column);
        # cols 0-63 are overwritten by the V-projection copies, col 65 unused.
        vcol = persist.tile([P, 1], BF16)
        nc.vector.memset(vcol, 1.0)
        nc.vector.tensor_copy(
            v_ext[:, :, :, 64:65],
            vcol[:, None, None, :].to_broadcast((P, 16, 8, 1)),
        )

        # ---- projections: Q, K (transposed out: [dims, S]), then V ----
        with tc.tile_pool(name="xtp", bufs=1) as xtp, \
             tc.tile_pool(name="wpool", bufs=2) as wpool, \
             tc.tile_pool(name="prps", bufs=4, space="PSUM") as prps:
            xt = xtp.tile([P, 8, S], BF16)
            nc.sync.dma_start(
                out=xt, in_=xt_e.ap().rearrange("(dc p) s -> p dc s", p=P)
            )
            for w_ext, dst_base in [(wq_e, 0), (wk_e, 4)]:
                w_sb = wpool.tile([P, 8, OG], BF16, tag="w")
                nc.sync.dma_start(
                    out=w_sb, in_=w_ext.ap().rearrange("(dc p) o -> p dc o", p=P)
                )
                for ob in range(4):
                    for scn in range(4):
                        ps = prps.tile([P, 512], F32, tag="projps")
                        for dc in range(8):
                            nc.tensor.matmul(
                                ps,
                                w_sb[:, dc, ob * P:(ob + 1) * P],
                                xt[:, dc, scn * 512:(scn + 1) * 512],
                                start=(dc == 0),
                                stop=(dc == 7),
                            )
                        nc.vector.tensor_copy(
                            qkraw[:, dst_base + ob, scn * 512:(scn + 1) * 512],
                            ps,
                        )
            # V projection: natural layout [s, dims] -> v_ext[:, sb, h, 0:64]
            w_sb = wpool.tile([P, 8, OG], BF16, tag="w")
            nc.sync.dma_start(
                out=w_sb, in_=wv_e.ap().rearrange("(dc p) o -> p dc o", p=P)
            )
            for sb in range(16):
                ps = prps.tile([P, 512], F32, tag="projps")
                for dc in range(8):
                    nc.tensor.matmul(
                        ps,
                        xt[:, dc, sb * P:(sb + 1) * P],
                        w_sb[:, dc, :],
                        start=(dc == 0),
                        stop=(dc == 7),
                    )
                nc.vector.tensor_copy(
                    v_ext[:, sb, :, 0:64],
                    ps.rearrange("p (h d) -> p h d", h=8),
                )

        # ---- RoPE on Q and K (in place) ----
        with tc.tile_pool(name="trig", bufs=1) as trig, \
             tc.tile_pool(name="rtmp", bufs=1) as rtmp:
            cos_s = trig.tile([P, S], BF16)
            sin_s = trig.tile([P, S], BF16)
            nc.sync.dma_start(out=cos_s, in_=cos_e.ap())
            nc.sync.dma_start(out=sin_s, in_=sin_e.ap())
            for pb in (0, 2, 4, 6):
                e_blk = qkraw[:, pb, :]
                o_blk = qkraw[:, pb + 1, :]
                t0 = rtmp.tile([P, S], BF16, tag="t0")
                t1 = rtmp.tile([P, S], BF16, tag="t1")
                t2 = rtmp.tile([P, S], BF16, tag="t2")
                nc.vector.tensor_mul(t0, e_blk, cos_s)
                nc.vector.tensor_mul(t1, o_blk, sin_s)
                nc.vector.tensor_mul(t2, e_blk, sin_s)
                # e' = e*cos - o*sin ; o' = e*sin + o*cos  (in place)
                nc.vector.tensor_mul(o_blk, o_blk, cos_s)  # o_blk := o*cos
                nc.vector.tensor_sub(e_blk, t0, t1)
                nc.vector.tensor_add(o_blk, t2, o_blk)
        # ---- repack to head-contiguous layout: rqh blk t holds the head
        # pair (2t, 2t+1) as 64 contiguous rows each ([evens|odds] per head),
        # so score matmuls contract K=64 in one shot. 32 small SBUF->SBUF
        # DMAs, hidden under the projection/RoPE tail.
        rqh = persist.tile([P, 8, S], BF16, tag="rqh")
        for t in range(8):
            side = 4 * (t // 4)          # 0 = Q blocks, 4 = K blocks
            p_loc = t % 4                # head pair within side
            be = side + 2 * (p_loc // 2)
            bo = be + 1
            for hh in range(2):          # head within pair
                h = 2 * p_loc + hh
                row = 32 * (h % 4)
                nc.sync.dma_start(
                    out=rqh[64 * hh:64 * hh + 32, t, :],
                    in_=qkraw[row:row + 32, be, :],
                )
                nc.sync.dma_start(
                    out=rqh[64 * hh + 32:64 * hh + 64, t, :],
                    in_=qkraw[row:row + 32, bo, :],
                )

        # ---- attention + incremental out-projection ----
        with tc.tile_pool(name="small", bufs=1) as small, \
             tc.tile_pool(name="wo_pool", bufs=1) as wo_pool, \
             tc.tile_pool(name="ppool", bufs=6) as ppool, \
             tc.tile_pool(name="opool", bufs=2) as opool, \
             tc.tile_pool(name="stg", bufs=4) as stgp, \
             tc.tile_pool(name="rbcp", bufs=3) as rbcp, \
             tc.tile_pool(name="yout", bufs=3) as yout, \
             tc.tile_pool(name="sps", bufs=2, space="PSUM") as spsp, \
             tc.tile_pool(name="avps", bufs=2, space="PSUM") as avpsp, \
             tc.tile_pool(name="yps", bufs=2, space="PSUM") as ypsp, \
             tc.tile_pool(name="rbdram", bufs=2, space="DRAM") as rbdram, \
             tc.tile_pool(name="ardram", bufs=2, space="DRAM") as ardram:
            mb_s = small.tile([P, 4, 512], BF16)
            nc.sync.dma_start(out=mb_s, in_=mb_e.ap())
            id_s = small.tile([P, P], BF16)
            nc.sync.dma_start(out=id_s, in_=id_e.ap())
            wo_s = wo_pool.tile([P, 4, D], BF16)
            nc.sync.dma_start(
                out=wo_s, in_=wo_e.ap().rearrange("(dc p) o -> p dc o", p=P)
            )

            for scn in range(4):
                o_chunk = opool.tile([P, 4, 512], BF16, tag="ochunk")
                nk = 4 * (scn + 1)
                for pr in range(4):
                    h0 = 2 * pr
                    h1 = 2 * pr + 1
                    av0 = avpsp.tile([65, 512], F32, tag="av", name="av0")
                    av1 = avpsp.tile([65, 512], F32, tag="av", name="av1")
                    av = [av0, av1]
                    for kb in range(nk):
                        sps = spsp.tile([P, 2, 512], F32, tag="sps")
                        diag = kb >= 4 * scn
                        for hh in range(2):
                            rows = slice(64 * hh, 64 * hh + 64)
                            nc.tensor.matmul(
                                sps[:, hh, :],
                                rqh[rows, 4 + pr, kb * P:(kb + 1) * P],
                                rqh[rows, pr, scn * 512:(scn + 1) * 512],
                                start=True,
                                stop=not diag,
                                tile_position=(64 * hh, 0),
                            )
                            if diag:
                                nc.tensor.matmul(
                                    sps[:, hh, :],
                                    id_s,
                                    mb_s[:, kb - 4 * scn, :],
                                    start=False,
                                    stop=True,
                                )
                        pt = ppool.tile([P, 2, 512], BF16, tag="pt")
                        nc.scalar.activation(
                            pt, sps, mybir.ActivationFunctionType.Exp, scale=0.125
                        )
                        for hh, h in enumerate((h0, h1)):
                            nc.tensor.matmul(
                                av[hh],
                                v_ext[:, kb, h, 0:65],
                                pt[:, hh, :],
                                start=(kb == 0),
                                stop=(kb == nk - 1),
                            )
                    # normalize: o_chunk[:, pr] rows 0-63 = head h0, 64-127 = h1
                    # 1/s as exp(-ln(s)) on the scalar engine: DVE reciprocal
                    # is an 8x-iterative op and costs 4.3us per 512-elem row
                    stg = stgp.tile([1, 1024], F32, tag="stg")
                    lns = stgp.tile([1, 1024], F32, tag="lns")
                    nc.scalar.activation(
                        lns[0:1, 0:512], av[0][64:65, :],
                        mybir.ActivationFunctionType.Ln,
                    )
                    nc.scalar.activation(
                        lns[0:1, 512:1024], av[1][64:65, :],
                        mybir.ActivationFunctionType.Ln,
                    )
                    nc.scalar.activation(
                        stg, lns, mybir.ActivationFunctionType.Exp, scale=-1.0
                    )
                    # broadcast the two recip rows across 64 partitions each:
                    # bounce through DRAM, read back with a stride-0 leading
                    # dim (partition-broadcast DMA)
                    rb_d = rbdram.tile([1, 1024], F32, tag="rbd")
                    nc.sync.dma_start(out=rb_d, in_=stg)
                    rbc_sb = rbcp.tile([P, 512], F32, tag="rbcsb")
                    nc.sync.dma_start(
                        out=rbc_sb[0:64, :],
                        in_=rb_d[0:1, 0:512].to_broadcast((64, 512)),
                    )
                    nc.sync.dma_start(
                        out=rbc_sb[64:128, :],
                        in_=rb_d[0:1, 512:1024].to_broadcast((64, 512)),
                    )
                    # copy O' out of PSUM right away (releases the AV
                    # accumulator banks for the next pair, keeping the PE
                    # fed), then normalize in place once the reciprocal
                    # broadcast lands
                    nc.vector.tensor_copy(o_chunk[0:64, pr, :], av[0][0:64, :])
                    nc.vector.tensor_copy(o_chunk[64:128, pr, :], av[1][0:64, :])
                    nc.vector.tensor_mul(
                        o_chunk[0:64, pr, :], o_chunk[0:64, pr, :],
                        rbc_sb[0:64, :],
                    )
                    nc.vector.tensor_mul(
                        o_chunk[64:128, pr, :], o_chunk[64:128, pr, :],
                        rbc_sb[64:128, :],
                    )
                # out-projection for this q-chunk -> DRAM bounce, then a
                # pairwise AllReduce of just this chunk (overlaps the next
                # chunk's attention), then the final y write for these rows
                arin = ardram.tile([512, D], F32, tag="arin")
                arout = ardram.tile([512, D], F32, tag="arout")
                for qb in range(4):
                    for oc in range(2):
                        yps = ypsp.tile([P, 512], F32, tag="yps")
                        for db in range(4):
                            nc.tensor.matmul(
                                yps,
                                o_chunk[:, db, qb * P:(qb + 1) * P],
                                wo_s[:, db, oc * 512:(oc + 1) * 512],
                                start=(db == 0),
                                stop=(db == 3),
                            )
                        yt = yout.tile([P, 512], F32, tag="yt")
                        nc.vector.tensor_copy(yt, yps)
                        nc.sync.dma_start(
                            out=arin[qb * P:(qb + 1) * P, oc * 512:(oc + 1) * 512],
                            in_=yt,
                        )
                nc.gpsimd.collective_compute(
                    "AllReduce",
                    mybir.AluOpType.add,
                    replica_groups=[[0, 1], [2, 3], [4, 5], [6, 7]],
                    ins=[arin.opt()],
                    outs=[arout.opt()],
                )
                nc.sync.dma_start(
                    out=y_ext.ap()[scn * 512:(scn + 1) * 512, :], in_=arout
                )

    if split_waits:
        _split_multi_waits(nc)
    return nc


def _host_inputs(x, Wq, Wk, Wv, Wo, token_positions):
    """Per-core input dicts. Host work is layout-only (transpose/slice/tables)."""
    half = DK // 2
    k = np.arange(1, half + 1, dtype=np.float64)
    inv_freq = THETA ** (-(2.0 * k - 2.0) / DK)  # [32]
    pos = np.asarray(token_positions).astype(np.float64)  # [S]
    ang = pos[None, :] * inv_freq[:, None]  # [32, S]
    cosT = np.ascontiguousarray(np.tile(np.cos(ang), (4, 1))).astype(BF)
    sinT = np.ascontiguousarray(np.tile(np.sin(ang), (4, 1))).astype(BF)

    # permuted Q/K dim order within a group: blk(4) x h4(4) x k(32);
    # blk 0/1 = heads 0-3 evens/odds, blk 2/3 = heads 4-7 evens/odds
    perm = np.empty(OG, dtype=np.int64)
    i = 0
    for blk in range(4):
        for h4 in range(4):
            for kk in range(32):
                perm[i] = 64 * (4 * (blk // 2) + h4) + 2 * kk + (blk % 2)
                i += 1

    pp = np.arange(P)[:, None]
    ff = np.arange(512)[None, :]
    mb = np.empty((P, 4, 512), dtype=np.float32)
    for r in range(4):
        mb[:, r, :] = np.where(ff >= 128 * r + pp, 0.0, NEG)
    mb = mb.astype(BF)
    ident = np.eye(P, dtype=np.float32).astype(BF)

    in_maps = []
    for c in range(8):
        b = c // 2
        g = c % 2
        gd = slice(g * OG, (g + 1) * OG)  # group's head dims among 1024
        gdim = np.arange(g * OG, (g + 1) * OG)
        xt = np.ascontiguousarray(x[b].T).astype(BF)  # [D, S]
        wqt = np.ascontiguousarray(Wq[gdim[perm], :].T).astype(BF)
        wkt = np.ascontiguousarray(Wk[gdim[perm], :].T).astype(BF)
        wvt = np.ascontiguousarray(Wv[gd, :].T).astype(BF)
        wot = np.ascontiguousarray(Wo[:, gd].T).astype(BF)
        in_maps.append({
            "xt": xt, "wqt": wqt, "wkt": wkt, "wvt": wvt, "wot": wot,
            "cosT": cosT, "sinT": sinT, "mb": mb, "ident": ident,
        })
    return in_maps


def kernel(x, Wq, Wk, Wv, Wo, token_positions, _trace=False):
    if "nc" not in _cache:
        _cache["nc"] = _build_nc()
    nc = _cache["nc"]
    in_maps = _host_inputs(x, Wq, Wk, Wv, Wo, token_positions)
    res = run_bass_kernel_spmd(
        nc, in_maps, core_ids=list(range(8)), trace=_trace
    )
    _cache["last_result"] = res
    out = np.empty((x.shape[0], S, D), dtype=np.float32)
    for b in range(x.shape[0]):
        out[b] = res.results[2 * b]["y"]
    return out


# revision 29
# speedup vs baseline: 1.9001x; 1.0395x over previous
"""Causal multi-head self-attention (RoPE) on 8 TRN2 NeuronCores.

Sharding: core c = (batch b = c//2, head-group g = c%2). Each core computes
QKV projections for its 8 heads on its batch, RoPE, causal attention in
transposed-score space (scores^T = [k_part, q_free]; softmax sums via a
ones-column appended to V), a partial out-projection over its 512 head dims,
then a pairwise AllReduce [[0,1],[2,3],[4,5],[6,7]] sums the two head-group
partials into the full output.

Shapes (hardcoded): x [4, 2048, 1024], Wq/Wk/Wv/Wo [1024, 1024],
token_positions [2048]. D_K=64, N_HEADS=16, THETA=10000.

Matmul operands are bf16 (PSUM accumulation in fp32); softmax statistics and
the final output stay fp32. Host work is layout-only: slicing, transposes,
RoPE cos/sin tables, masks.
"""
import copy
import sys

sys.path.insert(0, "/opt/trn_rl_repo")

import ml_dtypes
import numpy as np

import bass_rust
import concourse.bass as bass
import concourse.mybir as mybir
import concourse.tile as tile
from concourse.bass_utils import run_bass_kernel_spmd

P = 128
S = 2048
D = 1024
OG = 512          # head dims per group (8 heads x 64)
DK = 64
THETA = 10000.0
F32 = mybir.dt.float32
BF16 = mybir.dt.bfloat16
NEG = -1.0e30
BF = ml_dtypes.bfloat16

_cache = {}


def _split_multi_waits(nc, max_waits=1):
    """The staged walrus build rejects instructions carrying more than one
    attached sem-wait ("Too many sync wait commands"). Hoist excess waits
    into standalone single-wait EventSemaphore instructions just before the
    offending instruction (same engine, so semantics are identical)."""
    n_split = 0
    new_module = copy.replace(nc.m, functions=[])
    for function in nc.m.functions:
        new_function = copy.replace(function, blocks=[])
        new_function.set_allocations_from_list(function.allocations)
        for block in function.blocks:
            new_insts = []
            for inst in block.instructions:
                si = inst.sync_info
                if si is not None and len(si.on_wait) > max_waits:
                    waits = list(si.on_wait)
                    for j, w in enumerate(waits[:-max_waits]):
                        ev = bass_rust.InstEventSemaphore(
                            name=f"{inst.name}-wsplit{j}", ins=[], outs=[]
                        )
                        ev.engine = inst.engine
                        ev.sync_info = bass_rust.SyncInfo(on_wait=[w], on_update=[])
                        new_insts.append(ev)
                        n_split += 1
                    si.on_wait = waits[-max_waits:]
                new_insts.append(inst)
            new_block = copy.replace(block, instructions=new_insts)
            new_function.blocks.append(new_block)
        new_module.functions.append(new_function)
    nc.m = new_module
    return n_split


def _build_nc(split_waits=True):
    nc = bass.Bass(num_devices=8)

    xt_e = nc.declare_dram_parameter("xt", [D, S], BF16, isOutput=False)
    wq_e = nc.declare_dram_parameter("wqt", [D, OG], BF16, isOutput=False)
    wk_e = nc.declare_dram_parameter("wkt", [D, OG], BF16, isOutput=False)
    wv_e = nc.declare_dram_parameter("wvt", [D, OG], BF16, isOutput=False)
    wo_e = nc.declare_dram_parameter("wot", [OG, D], BF16, isOutput=False)
    cos_e = nc.declare_dram_parameter("cosT", [P, S], BF16, isOutput=False)
    sin_e = nc.declare_dram_parameter("sinT", [P, S], BF16, isOutput=False)
    mb_e = nc.declare_dram_parameter("mb", [P, 4, 512], BF16, isOutput=False)
    id_e = nc.declare_dram_parameter("ident", [P, P], BF16, isOutput=False)
    y_ext = nc.declare_dram_parameter("y", [S, D], F32, isOutput=True)

    ctx = tile.TileContext(nc)
    with ctx as tc, tc.tile_pool(name="persist", bufs=1) as persist:
        qkraw = persist.tile([P, 8, S], BF16, tag="qkraw")  # Q blocks 0-3, K 4-7
        v_ext = persist.tile([P, 16, 8, 66], BF16)
        # col 64 of every (sb, h) slot must be 1.0 (softmax-sum ones column);
        # cols 0-63 are overwritten by the V-projection copies, col 65 unused.
        vcol = persist.tile([P, 1], BF16)
        nc.vector.memset(vcol, 1.0)
        nc.vector.tensor_copy(
            v_ext[:, :, :, 64:65],
            vcol[:, None, None, :].to_broadcast((P, 16, 8, 1)),
        )

        # ---- projections: Q, K (transposed out: [dims, S]), then V ----
        with tc.tile_pool(name="xtp", bufs=1) as xtp, \
             tc.tile_pool(name="wpool", bufs=2) as wpool, \
             tc.tile_pool(name="prps", bufs=4, space="PSUM") as prps:
            xt = xtp.tile([P, 8, S], BF16)
            xt_src = xt_e.ap().rearrange("(dc p) s -> p dc s", p=P)
            first = True
            for w_ext, dst_base in [(wq_e, 0), (wk_e, 4)]:
                w_sb = wpool.tile([P, 8, OG], BF16, tag="w")
                w_src = w_ext.ap().rearrange("(dc p) o -> p dc o", p=P)
                if first:
                    # per-chunk loads so the first matmul starts ~3us in
                    for dc in range(8):
                        nc.sync.dma_start(out=w_sb[:, dc, :], in_=w_src[:, dc, :])
                        nc.sync.dma_start(out=xt[:, dc, :], in_=xt_src[:, dc, :])
                    first = False
                else:
                    nc.sync.dma_start(out=w_sb, in_=w_src)
                for ob in range(4):
                    for scn in range(4):
                        ps = prps.tile([P, 512], F32, tag="projps")
                        for dc in range(8):
                            nc.tensor.matmul(
                                ps,
                                w_sb[:, dc, ob * P:(ob + 1) * P],
                                xt[:, dc, scn * 512:(scn + 1) * 512],
                                start=(dc == 0),
                                stop=(dc == 7),
                            )
                        nc.vector.tensor_copy(
                            qkraw[:, dst_base + ob, scn * 512:(scn + 1) * 512],
                            ps,
                        )
            # V projection: natural layout [s, dims] -> v_ext[:, sb, h, 0:64]
            w_sb = wpool.tile([P, 8, OG], BF16, tag="w")
            nc.sync.dma_start(
                out=w_sb, in_=wv_e.ap().rearrange("(dc p) o -> p dc o", p=P)
            )
            for sb in range(16):
                ps = prps.tile([P, 512], F32, tag="projps")
                for dc in range(8):
                    nc.tensor.matmul(
                        ps,
                        xt[:, dc, sb * P:(sb + 1) * P],
                        w_sb[:, dc, :],
                        start=(dc == 0),
                        stop=(dc == 7),
                    )
                nc.vector.tensor_copy(
                    v_ext[:, sb, :, 0:64],
                    ps.rearrange("p (h d) -> p h d", h=8),
                )

        # ---- RoPE on Q and K (in place) ----
        with tc.tile_pool(name="trig", bufs=1) as trig, \
             tc.tile_pool(name="rtmp", bufs=1) as rtmp:
            cos_s = trig.tile([P, S], BF16)
            sin_s = trig.tile([P, S], BF16)
            nc.sync.dma_start(out=cos_s, in_=cos_e.ap())
            nc.sync.dma_start(out=sin_s, in_=sin_e.ap())
            # RoPE block order (0, 4, 2, 6): pair-0/1 Q then K blocks
            # first, with the head-contiguous repack DMAs interleaved per
            # rotated block, so the first score matmuls can start while the
            # remaining blocks are still rotating.
            rqh = persist.tile([P, 8, S], BF16, tag="rqh")
            for pb in (0, 4, 2, 6):
                e_blk = qkraw[:, pb, :]
                o_blk = qkraw[:, pb + 1, :]
                t0 = rtmp.tile([P, S], BF16, tag="t0")
                t1 = rtmp.tile([P, S], BF16, tag="t1")
                t2 = rtmp.tile([P, S], BF16, tag="t2")
                nc.vector.tensor_mul(t0, e_blk, cos_s)
                nc.vector.tensor_mul(t1, o_blk, sin_s)
                nc.vector.tensor_mul(t2, e_blk, sin_s)
                # e' = e*cos - o*sin ; o' = e*sin + o*cos  (in place)
                nc.vector.tensor_mul(o_blk, o_blk, cos_s)  # o_blk := o*cos
                nc.vector.tensor_sub(e_blk, t0, t1)
                nc.vector.tensor_add(o_blk, t2, o_blk)
                # repack this rotated block pair: rqh blk t holds head pair
                # (2t, 2t+1) as 64 contiguous rows each ([evens|odds])
                side = 4 * (pb // 4)         # 0 = Q blocks, 4 = K blocks
                for p_loc in (2 * ((pb % 4) // 2), 2 * ((pb % 4) // 2) + 1):
                    t = side + p_loc
                    for hh in range(2):      # head within pair
                        h = 2 * p_loc + hh
                        row = 32 * (h % 4)
                        nc.sync.dma_start(
                            out=rqh[64 * hh:64 * hh + 32, t, :],
                            in_=qkraw[row:row + 32, pb, :],
                        )
                        nc.sync.dma_start(
                            out=rqh[64 * hh + 32:64 * hh + 64, t, :],
                            in_=qkraw[row:row + 32, pb + 1, :],
                        )

        # ---- attention + incremental out-projection ----
        with tc.tile_pool(name="small", bufs=1) as small, \
             tc.tile_pool(name="wo_pool", bufs=1) as wo_pool, \
             tc.tile_pool(name="ppool", bufs=6) as ppool, \
             tc.tile_pool(name="opool", bufs=2) as opool, \
             tc.tile_pool(name="stg", bufs=4) as stgp, \
             tc.tile_pool(name="rbcp", bufs=3) as rbcp, \
             tc.tile_pool(name="yout", bufs=3) as yout, \
             tc.tile_pool(name="sps", bufs=2, space="PSUM") as spsp, \
             tc.tile_pool(name="avps", bufs=2, space="PSUM") as avpsp, \
             tc.tile_pool(name="yps", bufs=2, space="PSUM") as ypsp, \
             tc.tile_pool(name="rbdram", bufs=2, space="DRAM") as rbdram, \
             tc.tile_pool(name="ardram", bufs=2, space="DRAM") as ardram:
            mb_s = small.tile([P, 4, 512], BF16)
            nc.sync.dma_start(out=mb_s, in_=mb_e.ap())
            id_s = small.tile([P, P], BF16)
            nc.sync.dma_start(out=id_s, in_=id_e.ap())
            wo_s = wo_pool.tile([P, 4, D], BF16)
            nc.sync.dma_start(
                out=wo_s, in_=wo_e.ap().rearrange("(dc p) o -> p dc o", p=P)
            )

            for scn in range(4):
                o_chunk = opool.tile([P, 4, 512], BF16, tag="ochunk")
                nk = 4 * (scn + 1)
                for pr in range(4):
                    h0 = 2 * pr
                    h1 = 2 * pr + 1
                    av0 = avpsp.tile([65, 512], F32, tag="av", name="av0")
                    av1 = avpsp.tile([65, 512], F32, tag="av", name="av1")
                    av = [av0, av1]
                    for kb in range(nk):
                        sps = spsp.tile([P, 2, 512], F32, tag="sps")
                        diag = kb >= 4 * scn
                        for hh in range(2):
                            rows = slice(64 * hh, 64 * hh + 64)
                            nc.tensor.matmul(
                                sps[:, hh, :],
                                rqh[rows, 4 + pr, kb * P:(kb + 1) * P],
                                rqh[rows, pr, scn * 512:(scn + 1) * 512],
                                start=True,
                                stop=not diag,
                                tile_position=(64 * hh, 0),
                            )
                            if diag:
                                nc.tensor.matmul(
                                    sps[:, hh, :],
                                    id_s,
                                    mb_s[:, kb - 4 * scn, :],
                                    start=False,
                                    stop=True,
                                )
                        pt = ppool.tile([P, 2, 512], BF16, tag="pt")
                        nc.scalar.activation(
                            pt, sps, mybir.ActivationFunctionType.Exp, scale=0.125
                        )
                        for hh, h in enumerate((h0, h1)):
                            nc.tensor.matmul(
                                av[hh],
                                v_ext[:, kb, h, 0:65],
                                pt[:, hh, :],
                                start=(kb == 0),
                                stop=(kb == nk - 1),
                            )
                    # normalize: o_chunk[:, pr] rows 0-63 = head h0, 64-127 = h1
                    # pull the softmax sums out of PSUM with cheap DVE
                    # copies (so the AV banks free immediately), then
                    # 1/s = exp(-ln(s)) on the scalar engine: DVE reciprocal
                    # is an 8x-iterative op and costs 4.3us per 512-elem row
                    stg = stgp.tile([1, 1024], F32, tag="stg")
                    lns = stgp.tile([1, 1024], F32, tag="lns")
                    ssb = stgp.tile([1, 1024], F32, tag="ssb")
                    nc.vector.tensor_copy(ssb[0:1, 0:512], av[0][64:65, :])
                    nc.vector.tensor_copy(ssb[0:1, 512:1024], av[1][64:65, :])
                    nc.scalar.activation(
                        lns, ssb, mybir.ActivationFunctionType.Ln,
                    )
                    nc.scalar.activation(
                        stg, lns, mybir.ActivationFunctionType.Exp, scale=-1.0
                    )
                    # broadcast the two recip rows across 64 partitions each:
                    # bounce through DRAM, read back with a stride-0 leading
                    # dim (partition-broadcast DMA)
                    rb_d = rbdram.tile([1, 1024], F32, tag="rbd")
                    nc.sync.dma_start(out=rb_d, in_=stg)
                    rbc_sb = rbcp.tile([P, 512], F32, tag="rbcsb")
                    nc.sync.dma_start(
                        out=rbc_sb[0:64, :],
                        in_=rb_d[0:1, 0:512].to_broadcast((64, 512)),
                    )
                    nc.sync.dma_start(
                        out=rbc_sb[64:128, :],
                        in_=rb_d[0:1, 512:1024].to_broadcast((64, 512)),
                    )
                    # copy O' out of PSUM right away (releases the AV
                    # accumulator banks for the next pair, keeping the PE
                    # fed), then normalize in place once the reciprocal
                    # broadcast lands
                    nc.vector.tensor_copy(o_chunk[0:64, pr, :], av[0][0:64, :])
                    nc.vector.tensor_copy(o_chunk[64:128, pr, :], av[1][0:64, :])
                    nc.vector.tensor_mul(
                        o_chunk[0:64, pr, :], o_chunk[0:64, pr, :],
                        rbc_sb[0:64, :],
                    )
                    nc.vector.tensor_mul(
                        o_chunk[64:128, pr, :], o_chunk[64:128, pr, :],
                        rbc_sb[64:128, :],
                    )
                # out-projection for this q-chunk -> DRAM bounce, then a
                # pairwise AllReduce of just this chunk (overlaps the next
                # chunk's attention), then the final y write for these rows
                arin = ardram.tile([512, D], F32, tag="arin")
                arout = ardram.tile([512, D], F32, tag="arout")
                # the last chunk's AllReduce is fully exposed at the kernel
                # tail, so split it in half to shrink the tail
                bounds = [(0, 4)] if scn < 3 else [(0, 2), (2, 4)]
                for q0, q1 in bounds:
                    for qb in range(q0, q1):
                        for oc in range(2):
                            yps = ypsp.tile([P, 512], F32, tag="yps")
                            for db in range(4):
                                nc.tensor.matmul(
                                    yps,
                                    o_chunk[:, db, qb * P:(qb + 1) * P],
                                    wo_s[:, db, oc * 512:(oc + 1) * 512],
                                    start=(db == 0),
                                    stop=(db == 3),
                                )
                            yt = yout.tile([P, 512], F32, tag="yt")
                            nc.vector.tensor_copy(yt, yps)
                            nc.sync.dma_start(
                                out=arin[qb * P:(qb + 1) * P,
                                         oc * 512:(oc + 1) * 512],
                                in_=yt,
                            )
                    nc.gpsimd.collective_compute(
                        "AllReduce",
                        mybir.AluOpType.add,
                        replica_groups=[[0, 1], [2, 3], [4, 5], [6, 7]],
                        ins=[arin[q0 * P:q1 * P, :].opt()],
                        outs=[arout[q0 * P:q1 * P, :].opt()],
                    )
                    nc.sync.dma_start(
                        out=y_ext.ap()[scn * 512 + q0 * P:scn * 512 + q1 * P, :],
                        in_=arout[q0 * P:q1 * P, :],
                    )

    if split_waits:
        _split_multi_waits(nc)
    return nc


def _host_inputs(x, Wq, Wk, Wv, Wo, token_positions):
    """Per-core input dicts. Host work is layout-only (transpose/slice/tables)."""
    half = DK // 2
    k = np.arange(1, half + 1, dtype=np.float64)
    inv_freq = THETA ** (-(2.0 * k - 2.0) / DK)  # [32]
    pos = np.asarray(token_positions).astype(np.float64)  # [S]
    ang = pos[None, :] * inv_freq[:, None]  # [32, S]
    cosT = np.ascontiguousarray(np.tile(np.cos(ang), (4, 1))).astype(BF)
    sinT = np.ascontiguousarray(np.tile(np.sin(ang), (4, 1))).astype(BF)

    # permuted Q/K dim order within a group: blk(4) x h4(4) x k(32);
    # blk 0/1 = heads 0-3 evens/odds, blk 2/3 = heads 4-7 evens/odds
    perm = np.empty(OG, dtype=np.int64)
    i = 0
    for blk in range(4):
        for h4 in range(4):
            for kk in range(32):
                perm[i] = 64 * (4 * (blk // 2) + h4) + 2 * kk + (blk % 2)
                i += 1

    pp = np.arange(P)[:, None]
    ff = np.arange(512)[None, :]
    mb = np.empty((P, 4, 512), dtype=np.float32)
    for r in range(4):
        mb[:, r, :] = np.where(ff >= 128 * r + pp, 0.0, NEG)
    mb = mb.astype(BF)
    ident = np.eye(P, dtype=np.float32).astype(BF)

    in_maps = []
    for c in range(8):
        b = c // 2
        g = c % 2
        gd = slice(g * OG, (g + 1) * OG)  # group's head dims among 1024
        gdim = np.arange(g * OG, (g + 1) * OG)
        xt = np.ascontiguousarray(x[b].T).astype(BF)  # [D, S]
        wqt = np.ascontiguousarray(Wq[gdim[perm], :].T).astype(BF)
        wkt = np.ascontiguousarray(Wk[gdim[perm], :].T).astype(BF)
        wvt = np.ascontiguousarray(Wv[gd, :].T).astype(BF)
        wot = np.ascontiguousarray(Wo[:, gd].T).astype(BF)
        in_maps.append({
            "xt": xt, "wqt": wqt, "wkt": wkt, "wvt": wvt, "wot": wot,
            "cosT": cosT, "sinT": sinT, "mb": mb, "ident": ident,
        })
    return in_maps


def kernel(x, Wq, Wk, Wv, Wo, token_positions, _trace=False):
    if "nc" not in _cache:
        _cache["nc"] = _build_nc()
    nc = _cache["nc"]
    in_maps = _host_inputs(x, Wq, Wk, Wv, Wo, token_positions)
    res = run_bass_kernel_spmd(
        nc, in_maps, core_ids=list(range(8)), trace=_trace
    )
    _cache["last_result"] = res
    out = np.empty((x.shape[0], S, D), dtype=np.float32)
    for b in range(x.shape[0]):
        out[b] = res.results[2 * b]["y"]
    return out


# revision 31
# speedup vs baseline: 2.0368x; 1.0720x over previous
"""Causal multi-head self-attention (RoPE) on 8 TRN2 NeuronCores.

Sharding: core c = (batch b = c//2, head-group g = c%2). Each core computes
QKV projections for its 8 heads on its batch, RoPE, causal attention in
transposed-score space (scores^T = [k_part, q_free]; softmax sums via a
ones-column appended to V), a partial out-projection over its 512 head dims,
then a pairwise AllReduce [[0,1],[2,3],[4,5],[6,7]] sums the two head-group
partials into the full output.

Shapes (hardcoded): x [4, 2048, 1024], Wq/Wk/Wv/Wo [1024, 1024],
token_positions [2048]. D_K=64, N_HEADS=16, THETA=10000.

Matmul operands are bf16 (PSUM accumulation in fp32); softmax statistics and
the final output stay fp32. Host work is layout-only: slicing, transposes,
RoPE cos/sin tables, masks.
"""
import copy
import sys

sys.path.insert(0, "/opt/trn_rl_repo")

import ml_dtypes
import numpy as np

import bass_rust
import concourse.bass as bass
import concourse.mybir as mybir
import concourse.tile as tile
from concourse.bass_utils import run_bass_kernel_spmd

P = 128
S = 2048
D = 1024
OG = 512          # head dims per group (8 heads x 64)
DK = 64
THETA = 10000.0
F32 = mybir.dt.float32
BF16 = mybir.dt.bfloat16
NEG = -1.0e30
BF = ml_dtypes.bfloat16

_cache = {}


def _split_multi_waits(nc, max_waits=1):
    """The staged walrus build rejects instructions carrying more than one
    attached sem-wait ("Too many sync wait commands"). Hoist excess waits
    into standalone single-wait EventSemaphore instructions just before the
    offending instruction (same engine, so semantics are identical)."""
    n_split = 0
    new_module = copy.replace(nc.m, functions=[])
    for function in nc.m.functions:
        new_function = copy.replace(function, blocks=[])
        new_function.set_allocations_from_list(function.allocations)
        for block in function.blocks:
            new_insts = []
            for inst in block.instructions:
                si = inst.sync_info
                if si is not None and len(si.on_wait) > max_waits:
                    waits = list(si.on_wait)
                    for j, w in enumerate(waits[:-max_waits]):
                        ev = bass_rust.InstEventSemaphore(
                            name=f"{inst.name}-wsplit{j}", ins=[], outs=[]
                        )
                        ev.engine = inst.engine
                        ev.sync_info = bass_rust.SyncInfo(on_wait=[w], on_update=[])
                        new_insts.append(ev)
                        n_split += 1
                    si.on_wait = waits[-max_waits:]
                new_insts.append(inst)
            new_block = copy.replace(block, instructions=new_insts)
            new_function.blocks.append(new_block)
        new_module.functions.append(new_function)
    nc.m = new_module
    return n_split


def _build_nc(split_waits=True):
    nc = bass.Bass(num_devices=8)

    xt_e = nc.declare_dram_parameter("xt", [D, S], BF16, isOutput=False)
    wq_e = nc.declare_dram_parameter("wqt", [D, OG], BF16, isOutput=False)
    wk_e = nc.declare_dram_parameter("wkt", [D, OG], BF16, isOutput=False)
    wv_e = nc.declare_dram_parameter("wvt", [D, OG], BF16, isOutput=False)
    wo_e = nc.declare_dram_parameter("wot", [OG, D], BF16, isOutput=False)
    cos_e = nc.declare_dram_parameter("cosT", [P, S], BF16, isOutput=False)
    sin_e = nc.declare_dram_parameter("sinT", [P, S], BF16, isOutput=False)
    mb_e = nc.declare_dram_parameter("mb", [P, 4, 512], BF16, isOutput=False)
    id_e = nc.declare_dram_parameter("ident", [P, P], BF16, isOutput=False)
    y_ext = nc.declare_dram_parameter("y", [S, D], F32, isOutput=True)

    ctx = tile.TileContext(nc)
    with ctx as tc, tc.tile_pool(name="persist", bufs=1) as persist:
        qkraw = persist.tile([P, 8, S], BF16, tag="qkraw")  # Q blocks 0-3, K 4-7
        v_ext = persist.tile([P, 16, 8, 66], BF16)
        # col 64 of every (sb, h) slot must be 1.0 (softmax-sum ones column);
        # cols 0-63 are overwritten by the V-projection copies, col 65 unused.
        vcol = persist.tile([P, 1], BF16)
        nc.vector.memset(vcol, 1.0)
        nc.vector.tensor_copy(
            v_ext[:, :, :, 64:65],
            vcol[:, None, None, :].to_broadcast((P, 16, 8, 1)),
        )

        # ---- projections: Q, K (transposed out: [dims, S]), then V ----
        with tc.tile_pool(name="xtp", bufs=1) as xtp, \
             tc.tile_pool(name="wpool", bufs=2) as wpool, \
             tc.tile_pool(name="prps", bufs=4, space="PSUM") as prps:
            xt = xtp.tile([P, 8, S], BF16)
            xt_src = xt_e.ap().rearrange("(dc p) s -> p dc s", p=P)
            first = True
            for w_ext, dst_base in [(wq_e, 0), (wk_e, 4)]:
                w_sb = wpool.tile([P, 8, OG], BF16, tag="w")
                w_src = w_ext.ap().rearrange("(dc p) o -> p dc o", p=P)
                if first:
                    # per-chunk loads so the first matmul starts ~3us in
                    for dc in range(8):
                        nc.sync.dma_start(out=w_sb[:, dc, :], in_=w_src[:, dc, :])
                        nc.sync.dma_start(out=xt[:, dc, :], in_=xt_src[:, dc, :])
                    first = False
                else:
                    nc.sync.dma_start(out=w_sb, in_=w_src)
                for ob in range(4):
                    for scn in range(4):
                        ps = prps.tile([P, 512], F32, tag="projps")
                        for dc in range(8):
                            nc.tensor.matmul(
                                ps,
                                w_sb[:, dc, ob * P:(ob + 1) * P],
                                xt[:, dc, scn * 512:(scn + 1) * 512],
                                start=(dc == 0),
                                stop=(dc == 7),
                            )
                        nc.vector.tensor_copy(
                            qkraw[:, dst_base + ob, scn * 512:(scn + 1) * 512],
                            ps,
                        )
            # V projection: natural layout [s, dims] -> v_ext[:, sb, h, 0:64]
            w_sb = wpool.tile([P, 8, OG], BF16, tag="w")
            nc.sync.dma_start(
                out=w_sb, in_=wv_e.ap().rearrange("(dc p) o -> p dc o", p=P)
            )
            for sb in range(16):
                ps = prps.tile([P, 512], F32, tag="projps")
                for dc in range(8):
                    nc.tensor.matmul(
                        ps,
                        xt[:, dc, sb * P:(sb + 1) * P],
                        w_sb[:, dc, :],
                        start=(dc == 0),
                        stop=(dc == 7),
                    )
                nc.vector.tensor_copy(
                    v_ext[:, sb, :, 0:64],
                    ps.rearrange("p (h d) -> p h d", h=8),
                )

        # ---- RoPE on Q and K (in place) ----
        with tc.tile_pool(name="trig", bufs=1) as trig, \
             tc.tile_pool(name="rtmp", bufs=1) as rtmp:
            cos_s = trig.tile([P, S], BF16)
            sin_s = trig.tile([P, S], BF16)
            nc.sync.dma_start(out=cos_s, in_=cos_e.ap())
            nc.sync.dma_start(out=sin_s, in_=sin_e.ap())
            # RoPE block order (0, 4, 2, 6): pair-0/1 Q then K blocks
            # first, with the head-contiguous repack DMAs interleaved per
            # rotated block, so the first score matmuls can start while the
            # remaining blocks are still rotating.
            rqh = persist.tile([P, 8, S], BF16, tag="rqh")
            for pb in (0, 4, 2, 6):
                e_blk = qkraw[:, pb, :]
                o_blk = qkraw[:, pb + 1, :]
                t0 = rtmp.tile([P, S], BF16, tag="t0")
                t1 = rtmp.tile([P, S], BF16, tag="t1")
                t2 = rtmp.tile([P, S], BF16, tag="t2")
                nc.vector.tensor_mul(t0, e_blk, cos_s)
                nc.vector.tensor_mul(t1, o_blk, sin_s)
                nc.vector.tensor_mul(t2, e_blk, sin_s)
                # e' = e*cos - o*sin ; o' = e*sin + o*cos  (in place)
                nc.vector.tensor_mul(o_blk, o_blk, cos_s)  # o_blk := o*cos
                nc.vector.tensor_sub(e_blk, t0, t1)
                nc.vector.tensor_add(o_blk, t2, o_blk)
                # repack this rotated block pair: rqh blk t holds head pair
                # (2t, 2t+1) as 64 contiguous rows each ([evens|odds])
                side = 4 * (pb // 4)         # 0 = Q blocks, 4 = K blocks
                for p_loc in (2 * ((pb % 4) // 2), 2 * ((pb % 4) // 2) + 1):
                    t = side + p_loc
                    for hh in range(2):      # head within pair
                        h = 2 * p_loc + hh
                        row = 32 * (h % 4)
                        nc.sync.dma_start(
                            out=rqh[64 * hh:64 * hh + 32, t, :],
                            in_=qkraw[row:row + 32, pb, :],
                        )
                        nc.sync.dma_start(
                            out=rqh[64 * hh + 32:64 * hh + 64, t, :],
                            in_=qkraw[row:row + 32, pb + 1, :],
                        )

        # ---- attention + incremental out-projection ----
        with tc.tile_pool(name="small", bufs=1) as small, \
             tc.tile_pool(name="wo_pool", bufs=1) as wo_pool, \
             tc.tile_pool(name="ppool", bufs=6) as ppool, \
             tc.tile_pool(name="opool", bufs=2) as opool, \
             tc.tile_pool(name="stg", bufs=4) as stgp, \
             tc.tile_pool(name="rbcp", bufs=3) as rbcp, \
             tc.tile_pool(name="yout", bufs=3) as yout, \
             tc.tile_pool(name="sps", bufs=2, space="PSUM") as spsp, \
             tc.tile_pool(name="avps", bufs=2, space="PSUM") as avpsp, \
             tc.tile_pool(name="yps", bufs=2, space="PSUM") as ypsp, \
             tc.tile_pool(name="rbdram", bufs=2, space="DRAM") as rbdram, \
             tc.tile_pool(name="ardram", bufs=2, space="DRAM") as ardram:
            mb_s = small.tile([P, 4, 512], BF16)
            nc.sync.dma_start(out=mb_s, in_=mb_e.ap())
            id_s = small.tile([P, P], BF16)
            nc.sync.dma_start(out=id_s, in_=id_e.ap())
            wo_s = wo_pool.tile([P, 4, D], BF16)
            nc.sync.dma_start(
                out=wo_s, in_=wo_e.ap().rearrange("(dc p) o -> p dc o", p=P)
            )

            for scn in range(4):
                o_chunk = opool.tile([P, 4, 512], BF16, tag="ochunk")
                nk = 4 * (scn + 1)
                for pr in range(4):
                    h0 = 2 * pr
                    h1 = 2 * pr + 1
                    av0 = avpsp.tile([65, 512], F32, tag="av", name="av0")
                    av1 = avpsp.tile([65, 512], F32, tag="av", name="av1")
                    av = [av0, av1]
                    for kb in range(nk):
                        sps = spsp.tile([P, 2, 512], F32, tag="sps")
                        diag = kb >= 4 * scn
                        for hh in range(2):
                            rows = slice(64 * hh, 64 * hh + 64)
                            nc.tensor.matmul(
                                sps[:, hh, :],
                                rqh[rows, 4 + pr, kb * P:(kb + 1) * P],
                                rqh[rows, pr, scn * 512:(scn + 1) * 512],
                                start=True,
                                stop=True,
                                tile_position=(64 * hh, 0),
                            )
                        pt = ppool.tile([P, 2, 512], BF16, tag="pt")
                        nc.scalar.activation(
                            pt, sps, mybir.ActivationFunctionType.Exp, scale=0.125
                        )
                        if diag:
                            # zero the above-diagonal probabilities with a 0/1
                            # bf16 mask on the (otherwise idle) vector engine
                            r = kb - 4 * scn
                            for hh in range(2):
                                nc.vector.tensor_mul(
                                    pt[:, hh, :], pt[:, hh, :], mb_s[:, r, :]
                                )
                        for hh, h in enumerate((h0, h1)):
                            nc.tensor.matmul(
                                av[hh],
                                v_ext[:, kb, h, 0:65],
                                pt[:, hh, :],
                                start=(kb == 0),
                                stop=(kb == nk - 1),
                            )
                    # normalize: o_chunk[:, pr] rows 0-63 = head h0, 64-127 = h1
                    # pull the softmax sums out of PSUM with cheap DVE
                    # copies (so the AV banks free immediately), then
                    # 1/s = exp(-ln(s)) on the scalar engine: DVE reciprocal
                    # is an 8x-iterative op and costs 4.3us per 512-elem row
                    stg = stgp.tile([1, 1024], F32, tag="stg")
                    lns = stgp.tile([1, 1024], F32, tag="lns")
                    ssb = stgp.tile([1, 1024], F32, tag="ssb")
                    nc.vector.tensor_copy(ssb[0:1, 0:512], av[0][64:65, :])
                    nc.vector.tensor_copy(ssb[0:1, 512:1024], av[1][64:65, :])
                    nc.scalar.activation(
                        lns, ssb, mybir.ActivationFunctionType.Ln,
                    )
                    nc.scalar.activation(
                        stg, lns, mybir.ActivationFunctionType.Exp, scale=-1.0
                    )
                    # broadcast the two recip rows across 64 partitions each:
                    # bounce through DRAM, read back with a stride-0 leading
                    # dim (partition-broadcast DMA)
                    rb_d = rbdram.tile([1, 1024], F32, tag="rbd")
                    nc.sync.dma_start(out=rb_d, in_=stg)
                    rbc_sb = rbcp.tile([P, 512], F32, tag="rbcsb")
                    nc.sync.dma_start(
                        out=rbc_sb[0:64, :],
                        in_=rb_d[0:1, 0:512].to_broadcast((64, 512)),
                    )
                    nc.sync.dma_start(
                        out=rbc_sb[64:128, :],
                        in_=rb_d[0:1, 512:1024].to_broadcast((64, 512)),
                    )
                    # copy O' out of PSUM right away (releases the AV
                    # accumulator banks for the next pair, keeping the PE
                    # fed), then normalize in place once the reciprocal
                    # broadcast lands
                    nc.vector.tensor_copy(o_chunk[0:64, pr, :], av[0][0:64, :])
                    nc.vector.tensor_copy(o_chunk[64:128, pr, :], av[1][0:64, :])
                    nc.vector.tensor_mul(
                        o_chunk[0:64, pr, :], o_chunk[0:64, pr, :],
                        rbc_sb[0:64, :],
                    )
                    nc.vector.tensor_mul(
                        o_chunk[64:128, pr, :], o_chunk[64:128, pr, :],
                        rbc_sb[64:128, :],
                    )
                # out-projection for this q-chunk -> DRAM bounce, then a
                # pairwise AllReduce of just this chunk (overlaps the next
                # chunk's attention), then the final y write for these rows
                arin = ardram.tile([512, D], F32, tag="arin")
                arout = ardram.tile([512, D], F32, tag="arout")
                # the last chunk's AllReduce is fully exposed at the kernel
                # tail, so split it in half to shrink the tail
                bounds = [(0, 4)] if scn < 3 else [(0, 2), (2, 4)]
                for q0, q1 in bounds:
                    for qb in range(q0, q1):
                        for oc in range(2):
                            yps = ypsp.tile([P, 512], F32, tag="yps")
                            for db in range(4):
                                nc.tensor.matmul(
                                    yps,
                                    o_chunk[:, db, qb * P:(qb + 1) * P],
                                    wo_s[:, db, oc * 512:(oc + 1) * 512],
                                    start=(db == 0),
                                    stop=(db == 3),
                                )
                            yt = yout.tile([P, 512], F32, tag="yt")
                            nc.vector.tensor_copy(yt, yps)
                            nc.sync.dma_start(
                                out=arin[qb * P:(qb + 1) * P,
                                         oc * 512:(oc + 1) * 512],
                                in_=yt,
                            )
                    nc.gpsimd.collective_compute(
                        "AllReduce",
                        mybir.AluOpType.add,
                        replica_groups=[[0, 1], [2, 3], [4, 5], [6, 7]],
                        ins=[arin[q0 * P:q1 * P, :].opt()],
                        outs=[arout[q0 * P:q1 * P, :].opt()],
                    )
                    # gpsimd queue: this DMA waits on the AllReduce, and on
                    # the sync queue that wait head-of-line-blocks the
                    # attention-critical broadcast DMAs behind it
                    nc.gpsimd.dma_start(
                        out=y_ext.ap()[scn * 512 + q0 * P:scn * 512 + q1 * P, :],
                        in_=arout[q0 * P:q1 * P, :],
                    )

    if split_waits:
        _split_multi_waits(nc)
    return nc


def _host_inputs(x, Wq, Wk, Wv, Wo, token_positions):
    """Per-core input dicts. Host work is layout-only (transpose/slice/tables)."""
    half = DK // 2
    k = np.arange(1, half + 1, dtype=np.float64)
    inv_freq = THETA ** (-(2.0 * k - 2.0) / DK)  # [32]
    pos = np.asarray(token_positions).astype(np.float64)  # [S]
    ang = pos[None, :] * inv_freq[:, None]  # [32, S]
    cosT = np.ascontiguousarray(np.tile(np.cos(ang), (4, 1))).astype(BF)
    sinT = np.ascontiguousarray(np.tile(np.sin(ang), (4, 1))).astype(BF)

    # permuted Q/K dim order within a group: blk(4) x h4(4) x k(32);
    # blk 0/1 = heads 0-3 evens/odds, blk 2/3 = heads 4-7 evens/odds
    perm = np.empty(OG, dtype=np.int64)
    i = 0
    for blk in range(4):
        for h4 in range(4):
            for kk in range(32):
                perm[i] = 64 * (4 * (blk // 2) + h4) + 2 * kk + (blk % 2)
                i += 1

    pp = np.arange(P)[:, None]
    ff = np.arange(512)[None, :]
    mb = np.empty((P, 4, 512), dtype=np.float32)
    for r in range(4):
        mb[:, r, :] = np.where(ff >= 128 * r + pp, 1.0, 0.0)
    mb = mb.astype(BF)
    ident = np.eye(P, dtype=np.float32).astype(BF)

    in_maps = []
    for c in range(8):
        b = c // 2
        g = c % 2
        gd = slice(g * OG, (g + 1) * OG)  # group's head dims among 1024
        gdim = np.arange(g * OG, (g + 1) * OG)
        xt = np.ascontiguousarray(x[b].T).astype(BF)  # [D, S]
        wqt = np.ascontiguousarray(Wq[gdim[perm], :].T).astype(BF)
        wkt = np.ascontiguousarray(Wk[gdim[perm], :].T).astype(BF)
        wvt = np.ascontiguousarray(Wv[gd, :].T).astype(BF)
        wot = np.ascontiguousarray(Wo[:, gd].T).astype(BF)
        in_maps.append({
            "xt": xt, "wqt": wqt, "wkt": wkt, "wvt": wvt, "wot": wot,
            "cosT": cosT, "sinT": sinT, "mb": mb, "ident": ident,
        })
    return in_maps


def kernel(x, Wq, Wk, Wv, Wo, token_positions, _trace=False):
    if "nc" not in _cache:
        _cache["nc"] = _build_nc()
    nc = _cache["nc"]
    in_maps = _host_inputs(x, Wq, Wk, Wv, Wo, token_positions)
    res = run_bass_kernel_spmd(
        nc, in_maps, core_ids=list(range(8)), trace=_trace
    )
    _cache["last_result"] = res
    out = np.empty((x.shape[0], S, D), dtype=np.float32)
    for b in range(x.shape[0]):
        out[b] = res.results[2 * b]["y"]
    return out
